# revision 2
# baseline (speedup 1.0000x reference)
"""GAT (2-layer, 8 heads) + MLP on 8 Trainium2 NeuronCores — v2.

Node-major transforms + wide-row edge gathers (dst-sharded graph parallel):
  - hs1full[n] = [x@W1 | alpha_src1 | alpha_dst1] computed node-major,
    replicated on every core from the replicated x input.
  - Edge tiles use a GLOBAL window schedule (identical dst windows on all
    cores, chosen so each core's window edges pack into 4 groups x 128
    slots): segment ids are contiguous within the window, so alpha_dst
    loads and result stores are plain static-sliced DMAs (no indirect
    scatter), and only the 4 source-row gathers per tile are indirect.
  - h2cat shard = t2 @ [W2 | W2 a2s | W2 a2d] node-major, one 8-core
    AllGather of the [6272, 528] bf16 table, then layer-2 edge tiles.
  - MLP node-major on the shard, output stored transposed [2, 6272].
Timing: LAST_EXEC_NS is the steady-state per-execution time measured by
chained pipelined runs (difference quotient) — this excludes the network
round-trip latency of the axon device tunnel but includes all device work.
"""
import sys
import time

for p in ("/opt/trn_rl_repo",):
    if p not in sys.path:
        sys.path.append(p)

import numpy as np
import ml_dtypes
from dataclasses import dataclass

BF16 = ml_dtypes.bfloat16

N_CORES = 8
N_REAL = 50000
NSHARD = 6272  # 8 * 6272 = 50176 >= 50001
P = 128


@dataclass
class Cfg:
    n_real: int
    n_cores: int
    nshard: int
    T: int
    IN: int = 128
    A: int = 8
    CH1: int = 32
    CH2: int = 64
    G: int = 4

    @property
    def npad(self):
        return self.n_cores * self.nshard

    @property
    def C1(self):
        return self.A * self.CH1  # 256

    @property
    def C2(self):
        return self.A * self.CH2  # 512

    @property
    def K1(self):
        return self.C1 + self.A  # 264: [h1 | as1]

    @property
    def K2(self):
        return self.C2 + self.A  # 520: [h2 | as2]


# ---------------------------------------------------------------- host tiling


def window_schedule(deg, cfg):
    """deg: [n_cores, nshard] per-local-node degree. Greedy global windows:
    window [d0, d1) closes when any core's packing would exceed G groups of
    128 slots, or width hits 126. Returns list of (d0, width)."""
    G = cfg.G
    n_cores, nsh = deg.shape
    wins = []
    d0 = 0
    g_idx = np.zeros(n_cores, dtype=np.int64)
    fill = np.zeros(n_cores, dtype=np.int64)
    d = d0
    while d < nsh:
        k = deg[:, d]
        over = fill + k > P
        ng = g_idx + over
        if (ng >= G).any() or (d - d0) >= 126:
            wins.append((d0, d - d0))
            d0 = d
            g_idx[:] = 0
            fill[:] = 0
            over = k > P
            assert not over.any()
            ng = g_idx
        fill = np.where(over, k, fill + k)
        g_idx = ng
        d += 1
    wins.append((d0, nsh - d0))
    return wins


def build_tiles(src_sorted, dst_sorted, lo, hi, wins, cfg):
    """Pack one core's dst-sorted edges into the global windows.
    Returns srcs [T,P,G] i32, sids [T,P,G] f32, mr [T,G*P] f32."""
    G = cfg.G
    SENT = cfg.npad  # gather sentinel: explicit zero row of the table
    counts = np.bincount(dst_sorted - lo, minlength=hi - lo)
    starts = np.zeros(hi - lo + 1, dtype=np.int64)
    np.cumsum(counts, out=starts[1:])

    T = len(wins)
    srcs = np.full((T, P, G), SENT, dtype=np.int32)
    sids = np.full((T, P, G), 127.0, dtype=np.float32)
    mr = np.full((T, G, P), 127.0, dtype=np.float32)
    for t, (d0, w) in enumerate(wins):
        g_idx, fill = 0, 0
        for dl in range(d0, d0 + w):
            k = int(counts[dl])
            if k == 0:
                continue
            if fill + k > P:
                g_idx += 1
                fill = 0
            assert g_idx < G
            sid = dl - d0
            s0 = int(starts[dl])
            sl = slice(fill, fill + k)
            srcs[t, sl, g_idx] = src_sorted[s0:s0 + k]
            sids[t, sl, g_idx] = float(sid)
            mr[t, g_idx, sl] = float(sid)
            fill += k
    return srcs, sids, mr.reshape(T, G * P)


def host_prep(x, edge_index, cfg):
    n = cfg.n_real
    src = np.concatenate([np.asarray(edge_index[0]), np.arange(n)]).astype(np.int64)
    dst = np.concatenate([np.asarray(edge_index[1]), np.arange(n)]).astype(np.int64)
    order = np.argsort(dst, kind="stable")
    src_s = src[order].astype(np.int32)
    dst_s = dst[order].astype(np.int32)

    xpad = np.zeros((cfg.npad, cfg.IN), dtype=BF16)
    xpad[:n] = np.asarray(x, dtype=np.float32).astype(BF16)

    bounds = np.searchsorted(dst_s, np.arange(0, cfg.npad + 1, cfg.nshard))
    deg = np.zeros((cfg.n_cores, cfg.nshard), dtype=np.int64)
    for c in range(cfg.n_cores):
        lo, hi = c * cfg.nshard, (c + 1) * cfg.nshard
        e0, e1 = bounds[c], bounds[c + 1]
        deg[c] = np.bincount(dst_s[e0:e1] - lo, minlength=cfg.nshard)

    wins = window_schedule(deg, cfg)
    cfg.T = len(wins)

    metas = []
    for c in range(cfg.n_cores):
        lo, hi = c * cfg.nshard, (c + 1) * cfg.nshard
        e0, e1 = bounds[c], bounds[c + 1]
        srcs, sids, mr = build_tiles(src_s[e0:e1], dst_s[e0:e1], lo, hi,
                                     wins, cfg)
        T, G = cfg.T, cfg.G
        # device layouts: srcs/sids -> [P, T*G] ; mr -> [Tpad(partition), G*P]
        srcs_dev = np.ascontiguousarray(
            srcs.transpose(1, 0, 2).reshape(P, T * G))
        sids_dev = np.ascontiguousarray(
            sids.transpose(1, 0, 2).reshape(P, T * G))
        metas.append((srcs_dev, sids_dev, mr.astype(BF16)))
    return xpad, wins, metas


def prep_weights(W1, a1_src, a1_dst, W2, a2_src, a2_dst, Wm1, bm1, b2, Wm2, cfg):
    def blockdiag(a, ch):
        B = np.zeros((cfg.A * ch, cfg.A), dtype=np.float32)
        for h in range(cfg.A):
            B[h * ch:(h + 1) * ch, h] = a[h]
        return B

    W1 = np.asarray(W1, np.float32)
    W2 = np.asarray(W2, np.float32)
    Wm1 = np.asarray(Wm1, np.float32)
    # [W | W bd(a_src) | W bd(a_dst)] — device uses cols [0:C+A) for the
    # gather table and cols [C+A:C+2A) for the shard-local alpha_dst table
    W1aug = np.concatenate(
        [W1, W1 @ blockdiag(np.asarray(a1_src, np.float32), cfg.CH1),
         W1 @ blockdiag(np.asarray(a1_dst, np.float32), cfg.CH1)], axis=1)
    W2aug = np.concatenate(
        [W2, W2 @ blockdiag(np.asarray(a2_src, np.float32), cfg.CH2),
         W2 @ blockdiag(np.asarray(a2_dst, np.float32), cfg.CH2)], axis=1)
    bm1p = np.asarray(bm1, np.float32) + np.asarray(b2, np.float32) @ Wm1
    return dict(
        W1aug=W1aug.astype(BF16),
        W2aug=W2aug.astype(BF16),
        Wm1=(Wm1 / cfg.A).astype(np.float32),  # folds the head-mean 1/8
        bm1=bm1p.reshape(-1, 1).astype(np.float32),
        Wm2=np.asarray(Wm2, np.float32),
    )


# ------------------------------------------------------------- device program

AMP_REPS = 1
DEBUG_OUTPUTS = False


def build_program(cfg, wins):
    from concourse import bass, bacc, mybir
    import concourse.tile as tile

    f32 = mybir.dt.float32
    bf16 = mybir.dt.bfloat16
    i32 = mybir.dt.int32
    A, C1, C2, K1, K2, G = cfg.A, cfg.C1, cfg.C2, cfg.K1, cfg.K2, cfg.G
    NSH, NPAD, T = cfg.nshard, cfg.npad, cfg.T
    NB = NSH // P       # 49 node blocks per shard
    SB = 512            # node-major superblock

    nc = bacc.Bacc("TRN2", target_bir_lowering=False, debug=False,
                   num_devices=cfg.n_cores, num_swdge_queues=4)

    xpad = nc.dram_tensor("xpad", [NPAD, cfg.IN], bf16, kind="ExternalInput")
    xshard = nc.dram_tensor("xshard", [NSH, cfg.IN], bf16,
                            kind="ExternalInput")
    srcs_d = nc.dram_tensor("srcs", [P, T * G], i32, kind="ExternalInput")
    sids_d = nc.dram_tensor("sids", [P, T * G], f32, kind="ExternalInput")
    mr_d = nc.dram_tensor("mr", [T, G * P], bf16, kind="ExternalInput")
    W1aug_d = nc.dram_tensor("W1aug", [cfg.IN, K1 + A], bf16,
                             kind="ExternalInput")
    W2aug_d = nc.dram_tensor("W2aug", [C1, K2 + A], bf16,
                             kind="ExternalInput")
    Wm1_d = nc.dram_tensor("Wm1", [64, 64], f32, kind="ExternalInput")
    bm1_d = nc.dram_tensor("bm1", [64, 1], f32, kind="ExternalInput")
    Wm2_d = nc.dram_tensor("Wm2", [64, 2], f32, kind="ExternalInput")

    hs1 = nc.dram_tensor("hs1", [NPAD + P, K1], bf16, kind="Internal")
    ad1t = nc.dram_tensor("ad1t", [NSH + P, A], bf16, kind="Internal")
    ad2t = nc.dram_tensor("ad2t", [NSH + P, A], bf16, kind="Internal")
    t2shard = nc.dram_tensor("t2shard", [NSH, C1], bf16, kind="Internal")
    h2shard = nc.dram_tensor("h2shard", [NSH, K2], bf16, kind="Internal")
    h2full = nc.dram_tensor("h2full", [NPAD + P, K2], bf16, kind="Internal",
                            addr_space="Shared")
    h2m = nc.dram_tensor("h2m", [NSH, 64], f32, kind="Internal")
    outf = nc.dram_tensor("outf", [2, NSH], f32, kind="ExternalOutput")
    if DEBUG_OUTPUTS:
        dbg_hs1 = nc.dram_tensor("dbg_hs1", [1024, K1], bf16,
                                 kind="ExternalOutput")
        dbg_ad1 = nc.dram_tensor("dbg_ad1", [NSH, A], bf16,
                                 kind="ExternalOutput")
        dbg_t2 = nc.dram_tensor("dbg_t2", [NSH, C1], bf16,
                                kind="ExternalOutput")
        dbg_h2s = nc.dram_tensor("dbg_h2s", [NSH, K2], bf16,
                                 kind="ExternalOutput")
        dbg_h2m = nc.dram_tensor("dbg_h2m", [NSH, 64], f32,
                                 kind="ExternalOutput")

    EXP = mybir.ActivationFunctionType.Exp
    RELU = mybir.ActivationFunctionType.Relu
    ABS = mybir.ActivationFunctionType.Abs
    EQ = mybir.AluOpType.is_equal
    MUL = mybir.AluOpType.mult
    ADD = mybir.AluOpType.add

    with tile.TileContext(nc) as tc:
        with (
            tc.tile_pool(name="const", bufs=1) as cp,
            tc.tile_pool(name="work", bufs=4) as wp,
        ):
            # ---- constants ----
            iota = cp.tile([P, 1], i32, tag="iotai")
            nc.gpsimd.iota(iota[:], pattern=[[0, 1]], base=0,
                           channel_multiplier=1)
            iotaf = cp.tile([P, 1], f32, tag="iotaf")
            nc.vector.tensor_copy(iotaf[:], iota[:])
            iotar = cp.tile([P, P], i32, tag="iotari")
            nc.gpsimd.iota(iotar[:], pattern=[[1, P]], base=0,
                           channel_multiplier=0)
            iotarf = cp.tile([P, P], f32, tag="iotarf")
            nc.vector.tensor_copy(iotarf[:], iotar[:])
            ones1 = cp.tile([1, P], bf16, tag="ones1")
            nc.gpsimd.memset(ones1[:], 1.0)
            zrow = cp.tile([P, K2], bf16, tag="zrow")
            nc.gpsimd.memset(zrow[:], 0.0)
            ident = cp.tile([P, P], f32, tag="ident")
            from concourse.masks import make_identity
            make_identity(nc, ident[:])

            w1aug = cp.tile([cfg.IN, K1 + A], bf16, tag="w1aug")
            nc.sync.dma_start(w1aug[:], W1aug_d[:])
            w2aug = [cp.tile([P, K2 + A], bf16, tag=f"w2aug{b}",
                             name=f"w2aug{b}") for b in range(2)]
            for b in range(2):
                nc.sync.dma_start(w2aug[b][:], W2aug_d[b * P:(b + 1) * P, :])
            wm1 = cp.tile([64, 64], f32, tag="wm1")
            nc.sync.dma_start(wm1[:], Wm1_d[:])
            bm1 = cp.tile([64, 1], f32, tag="bm1")
            nc.sync.dma_start(bm1[:], bm1_d[:])
            wm2 = cp.tile([64, 2], f32, tag="wm2")
            nc.sync.dma_start(wm2[:], Wm2_d[:])

            srcs = cp.tile([P, T * G], i32, tag="srcs")
            nc.sync.dma_start(srcs[:], srcs_d[:])
            sids = cp.tile([P, T * G], f32, tag="sids")
            nc.sync.dma_start(sids[:], sids_d[:])

            # zero pad rows of the gather tables (sentinel row NPAD..NPAD+P)
            nc.sync.dma_start(hs1[NPAD:NPAD + P, :], zrow[:, 0:K1])
            nc.sync.dma_start(h2full[NPAD:NPAD + P, :], zrow[:, 0:K2])
            nc.sync.dma_start(ad1t[NSH:NSH + P, :], zrow[:, 0:A])
            nc.sync.dma_start(ad2t[NSH:NSH + P, :], zrow[:, 0:A])

            for _rep in range(AMP_REPS):
              # ---- phase A0: ad1t = xshard @ W1ad (shard-local rows) ----
              with tc.tile_pool(name="psA0", bufs=4, space="PSUM") as psA0:
                  done = 0
                  while done < NSH:
                      cur = min(SB, NSH - done)
                      nbl = cur // P
                      xsT = wp.tile([P, cur], bf16, tag=f"a0_xsT{cur}")
                      nc.sync.dma_start(xsT[:], xshard[done:done + cur, :],
                                        transpose=True)
                      adc = wp.tile([P, nbl * A], bf16, tag=f"a0_adc{cur}")
                      for nb in range(nbl):
                          ps = psA0.tile([P, A], f32, tag="a0_ps")
                          nc.tensor.matmul(
                              ps[:], lhsT=xsT[:, nb * P:(nb + 1) * P],
                              rhs=w1aug[:, K1:K1 + A],
                              start=True, stop=True)
                          nc.any.tensor_copy(adc[:, nb * A:(nb + 1) * A],
                                             ps[:])
                      nc.sync.dma_start(
                          ad1t[done:done + cur, :].rearrange(
                              "(b p) c -> p b c", p=P),
                          adc[:].rearrange("p (b c) -> p b c", b=nbl))
                      done += cur

              # ---- phase A1: hs1[n] = x @ [W1 | W1 a1s | W1 a1d], all nodes
              with tc.tile_pool(name="psA1", bufs=4, space="PSUM") as psA:
                  for sb in range(NPAD // SB):
                      xT = wp.tile([P, SB], bf16, tag="a1_xT")
                      nc.sync.dma_start(xT[:], xpad[sb * SB:(sb + 1) * SB, :],
                                        transpose=True)
                      hcat = wp.tile([P, 4 * K1], bf16, tag="a1_hcat")
                      for nb in range(4):
                          ps = psA.tile([P, K1], f32, tag="a1_ps")
                          nc.tensor.matmul(ps[:], lhsT=xT[:, nb * P:(nb + 1) * P],
                                           rhs=w1aug[:, 0:K1],
                                           start=True, stop=True)
                          nc.any.tensor_copy(hcat[:, nb * K1:(nb + 1) * K1],
                                             ps[:])
                      nc.sync.dma_start(
                          hs1[sb * SB:(sb + 1) * SB, :].rearrange(
                              "(b p) c -> p b c", p=P),
                          hcat[:].rearrange("p (b c) -> p b c", b=4))

              def dbg_copy(dst, src, rows, cols, dt):
                  for i in range(rows // P):
                      c = wp.tile([P, cols], dt, tag=f"dbgc{cols}{dt}")
                      nc.sync.dma_start(c[:], src[i * P:(i + 1) * P, 0:cols])
                      nc.sync.dma_start(dst[i * P:(i + 1) * P, :], c[:])

              if DEBUG_OUTPUTS:
                  dbg_copy(dbg_hs1, hs1, 1024, K1, bf16)
                  dbg_copy(dbg_ad1, ad1t, NSH, A, bf16)

              # ---- edge-tile phase (shared between the two GAT layers) ----
              def edge_prefetch(layer, t):
                  KW = K1 if layer == 1 else K2
                  tbl = hs1 if layer == 1 else h2full
                  d0, w = wins[t]
                  gt = wp.tile([P, G * KW], bf16, tag=f"g{layer}",
                               name=f"g{layer}")
                  for g in range(G):
                      nc.gpsimd.indirect_dma_start(
                          out=gt[:, g * KW:(g + 1) * KW],
                          out_offset=None, in_=tbl[:],
                          in_offset=bass.IndirectOffsetOnAxis(
                              ap=srcs[:, t * G + g:t * G + g + 1], axis=0))
                  # alpha_dst rows for this window: shard-local rows d0..d0+P
                  adx = wp.tile([P, A], bf16, tag="adx")
                  adt = ad1t if layer == 1 else ad2t
                  nc.sync.dma_start(adx[:], adt[d0:d0 + P, :])
                  mr = wp.tile([1, G * P], bf16, tag="mrt")
                  nc.sync.dma_start(mr[:], mr_d[t:t + 1, :])
                  return dict(gt=gt, adx=adx, mr=mr, t=t)

              def edge_compute(layer, st, pools):
                  gt, adx, t = st["gt"], st["adx"], st["t"]
                  KW = K1 if layer == 1 else K2
                  CO = C1 if layer == 1 else C2
                  CH = cfg.CH1 if layer == 1 else cfg.CH2
                  d0, w = wins[t]

                  # segb[p, g*P+s] = sid of slot s in group g (broadcast rows)
                  segb = pools["seg"].tile([P, G * P], f32, tag="segps")
                  nc.tensor.matmul(segb[:], lhsT=ones1[:], rhs=st["mr"][:],
                                   start=True, stop=True)
                  # Eg[p, g*P+s] = (p == sid(g,s)) ; lhsT for alpha_dst expand
                  Eg = wp.tile([P, G * P], bf16, tag="Eg")
                  nc.vector.tensor_tensor(
                      out=Eg[:], in0=iotaf[:].to_broadcast((P, G * P)),
                      in1=segb[:], op=EQ)
                  # ETg[p, g*P+s'] = (sid_of_slot_p(g) == s') ; aggregation
                  ETg = wp.tile([P, G * P], bf16, tag="ETg")
                  sl = sids[:, t * G:(t + 1) * G]
                  in0 = bass.AP(sl.tensor, sl.offset,
                                [list(sl.ap[0]), [1, G], [0, P]])
                  in1 = bass.AP(iotarf.tensor, iotarf[:].offset,
                                [list(iotarf[:].ap[0]), [0, G], [1, P]])
                  out = bass.AP(ETg.tensor, ETg[:].offset,
                                [list(ETg[:].ap[0]), [P, G], [1, P]])
                  nc.vector.tensor_tensor(out=out, in0=in0, in1=in1, op=EQ)

                  # attention logits: alpha_src (gathered cols) + Eg @ adx
                  att = pools["att"].tile([P, G * A], f32, tag="attps")
                  for g in range(G):
                      nc.tensor.matmul(att[:, g * A:(g + 1) * A],
                                       lhsT=Eg[:, g * P:(g + 1) * P],
                                       rhs=adx[:], start=True, stop=True,
                                       skip_group_check=True)
                  asv = bass.AP(gt.tensor, gt[:].offset + CO,
                                [list(gt[:].ap[0]), [KW, G], [1, A]])
                  ex = wp.tile([P, G * A], f32, tag="ex")
                  exv = ex[:].rearrange("p (g a) -> p g a", g=G)
                  nc.vector.tensor_tensor(out=exv, in0=asv,
                                          in1=att[:].rearrange(
                                              "p (g a) -> p g a", g=G),
                                          op=ADD)
                  # leaky-relu as 0.6x + 0.4|x|, then exp
                  ab = wp.tile([P, G * A], f32, tag="ab")
                  nc.scalar.activation(ab[:], ex[:], ABS, scale=0.4)
                  nc.vector.scalar_tensor_tensor(
                      out=ex[:], in0=ex[:], scalar=0.6, in1=ab[:],
                      op0=MUL, op1=ADD)
                  nc.scalar.activation(ex[:], ex[:], EXP)

                  # M per group: [ex (A) | msg (CO)] ; msg = h * ex_broadcast
                  M = wp.tile([P, G * (A + CO)], bf16, tag=f"M{layer}",
                              name=f"M{layer}")
                  mex = bass.AP(M.tensor, M[:].offset,
                                [list(M[:].ap[0]), [A + CO, G], [1, A]])
                  nc.any.tensor_copy(mex, ex[:].rearrange(
                      "p (g a) -> p g a", g=G))
                  for g in range(G):
                      nc.vector.tensor_tensor(
                          out=M[:, g * (A + CO) + A:(g + 1) * (A + CO)]
                              .rearrange("p (h c) -> p h c", h=A),
                          in0=gt[:, g * KW:g * KW + CO]
                              .rearrange("p (h c) -> p h c", h=A),
                          in1=ex[:, g * A:(g + 1) * A][:, :, None]
                              .to_broadcast((P, A, CH)), op=MUL)

                  # aggregate: [denom | numer] += ETg_g^T @ M_g
                  if layer == 1:
                      pab = pools["pab"].tile([P, A + CO], f32, tag="pab")
                      for g in range(G):
                          nc.tensor.matmul(pab[:],
                                           lhsT=ETg[:, g * P:(g + 1) * P],
                                           rhs=M[:, g * (A + CO):
                                                 (g + 1) * (A + CO)],
                                           start=(g == 0), stop=(g == G - 1))
                      den = pab[:, 0:A]
                      num = pab[:, A:A + CO]
                  else:
                      HCO = CO // 2
                      pa = pools["pab"].tile([P, A + HCO], f32, tag="paL2")
                      pb = pools["pb"].tile([P, HCO], f32, tag="pbL2")
                      for g in range(G):
                          nc.tensor.matmul(pa[:],
                                           lhsT=ETg[:, g * P:(g + 1) * P],
                                           rhs=M[:, g * (A + CO):
                                                 g * (A + CO) + A + HCO],
                                           start=(g == 0), stop=(g == G - 1))
                          nc.tensor.matmul(pb[:],
                                           lhsT=ETg[:, g * P:(g + 1) * P],
                                           rhs=M[:, g * (A + CO) + A + HCO:
                                                 (g + 1) * (A + CO)],
                                           start=(g == 0), stop=(g == G - 1))
                      den = pa[:, 0:A]
                      num = None

                  # +eps so empty segments (zero-degree pad nodes) yield 0,
                  # not 0*inf=NaN — NaN rows would poison the Eg@adx matmul
                  # of later windows via 0*NaN.
                  r = wp.tile([P, A], f32, tag="r")
                  nc.vector.tensor_scalar(
                      out=r[:], in0=den, scalar1=1e-30, scalar2=None,
                      op0=ADD)
                  nc.vector.reciprocal(r[:], r[:])
                  if layer == 1:
                      h1r = wp.tile([P, C1], bf16, tag="h1r")
                      nc.vector.tensor_tensor(
                          out=h1r[:].rearrange("p (h c) -> p h c", h=A),
                          in0=num.rearrange("p (h c) -> p h c", h=A),
                          in1=r[:, :, None].to_broadcast((P, A, CH)), op=MUL)
                      nc.scalar.activation(h1r[:], h1r[:], RELU)
                      nc.sync.dma_start(t2shard[d0:d0 + w, :], h1r[0:w, :])
                  else:
                      tmp = wp.tile([P, C2], f32, tag="tmp2")
                      nc.vector.tensor_tensor(
                          out=tmp[:, 0:HCO].rearrange("p (h c) -> p h c",
                                                      h=A // 2),
                          in0=pa[:, A:A + HCO].rearrange("p (h c) -> p h c",
                                                         h=A // 2),
                          in1=r[:, 0:A // 2][:, :, None]
                              .to_broadcast((P, A // 2, CH)), op=MUL)
                      nc.vector.tensor_tensor(
                          out=tmp[:, HCO:CO].rearrange("p (h c) -> p h c",
                                                       h=A // 2),
                          in0=pb[:].rearrange("p (h c) -> p h c", h=A // 2),
                          in1=r[:, A // 2:A][:, :, None]
                              .to_broadcast((P, A // 2, CH)), op=MUL)
                      o2 = wp.tile([P, 64], f32, tag="o2")
                      cview = bass.AP(tmp.tensor, tmp[:].offset,
                                      [list(tmp[:].ap[0]), [1, 64], [64, A]])
                      nc.vector.tensor_reduce(
                          out=o2[:], in_=cview, axis=mybir.AxisListType.X,
                          op=ADD)
                      nc.sync.dma_start(h2m[d0:d0 + w, :], o2[0:w, :])

              # ---- phase I: layer-1 edge tiles ----
              with (
                  tc.tile_pool(name="psseg1", bufs=2, space="PSUM") as pseg,
                  tc.tile_pool(name="psatt1", bufs=2, space="PSUM") as patt,
                  tc.tile_pool(name="pspab1", bufs=2, space="PSUM") as ppab,
              ):
                  pools = dict(seg=pseg, att=patt, pab=ppab, pb=None)
                  pend = []
                  for t in range(T):
                      pend.append(edge_prefetch(1, t))
                      if len(pend) > 2:
                          edge_compute(1, pend.pop(0), pools)
                  for st in pend:
                      edge_compute(1, st, pools)

              if DEBUG_OUTPUTS:
                  dbg_copy(dbg_t2, t2shard, NSH, C1, bf16)

              # ---- phase A2: h2cat = t2 @ [W2 | W2 a2s | W2 a2d] (shard) ---
              with tc.tile_pool(name="psA2", bufs=4, space="PSUM") as psA2:
                  done = 0
                  while done < NSH:
                      cur = min(SB, NSH - done)
                      nbl = cur // P
                      t2T = [wp.tile([P, cur], bf16, tag=f"a2_t2T{b}_{cur}",
                                     name=f"a2_t2T{b}_{cur}")
                             for b in range(2)]
                      for b in range(2):
                          nc.sync.dma_start(
                              t2T[b][:],
                              t2shard[done:done + cur, b * P:(b + 1) * P],
                              transpose=True)
                      KF = K2 + A  # 528
                      hcat = wp.tile([P, nbl * KF], bf16, tag=f"a2_hcat{cur}")
                      for nb in range(nbl):
                          psa = psA2.tile([P, KF // 2], f32, tag="a2_psa")
                          psb = psA2.tile([P, KF // 2], f32, tag="a2_psb")
                          for b in range(2):
                              nc.tensor.matmul(
                                  psa[:], lhsT=t2T[b][:, nb * P:(nb + 1) * P],
                                  rhs=w2aug[b][:, 0:KF // 2],
                                  start=(b == 0), stop=(b == 1))
                              nc.tensor.matmul(
                                  psb[:], lhsT=t2T[b][:, nb * P:(nb + 1) * P],
                                  rhs=w2aug[b][:, KF // 2:KF],
                                  start=(b == 0), stop=(b == 1))
                          nc.any.tensor_copy(
                              hcat[:, nb * KF:nb * KF + KF // 2], psa[:])
                          nc.any.tensor_copy(
                              hcat[:, nb * KF + KF // 2:(nb + 1) * KF],
                              psb[:])
                      hc = hcat[:, 0:nbl * KF]
                      nc.sync.dma_start(
                          h2shard[done:done + cur, :].rearrange(
                              "(b p) c -> p b c", p=P),
                          bass.AP(hcat.tensor, hc.offset,
                                  [list(hc.ap[0]), [KF, nbl], [1, K2]]))
                      nc.sync.dma_start(
                          ad2t[done:done + cur, :].rearrange(
                              "(b p) c -> p b c", p=P),
                          bass.AP(hcat.tensor, hc.offset + K2,
                                  [list(hc.ap[0]), [KF, nbl], [1, A]]))
                      done += cur

              # ---- AllGather the h2cat node table ----
              if cfg.n_cores > 1:
                  nc.gpsimd.collective_compute(
                      "AllGather", mybir.AluOpType.bypass,
                      replica_groups=[list(range(cfg.n_cores))],
                      ins=[h2shard[0:NSH, :].opt()],
                      outs=[h2full[0:NPAD, :].opt()])
              else:
                  for i in range(NB):
                      cpt = wp.tile([P, K2], bf16, tag="cpt")
                      nc.sync.dma_start(cpt[:], h2shard[i * P:(i + 1) * P, :])
                      nc.sync.dma_start(h2full[i * P:(i + 1) * P, :], cpt[:])

              # ---- phase II: layer-2 edge tiles ----
              with (
                  tc.tile_pool(name="psseg2", bufs=2, space="PSUM") as pseg,
                  tc.tile_pool(name="psatt2", bufs=2, space="PSUM") as patt,
                  tc.tile_pool(name="pspa2", bufs=2, space="PSUM") as ppa,
                  tc.tile_pool(name="pspb2", bufs=2, space="PSUM") as ppb,
              ):
                  pools = dict(seg=pseg, att=patt, pab=ppa, pb=ppb)
                  pend = []
                  for t in range(T):
                      pend.append(edge_prefetch(2, t))
                      if len(pend) > 2:
                          edge_compute(2, pend.pop(0), pools)
                  for st in pend:
                      edge_compute(2, st, pools)

              if DEBUG_OUTPUTS:
                  dbg_copy(dbg_h2s, h2shard, NSH, K2, bf16)
                  dbg_copy(dbg_h2m, h2m, NSH, 64, f32)

              # ---- phase III: MLP node-major over the shard ----
              with tc.tile_pool(name="ps3", bufs=2, space="PSUM") as ps3:
                  for i in range(NB):
                      hm = wp.tile([P, 64], f32, tag="p3_hm")
                      nc.sync.dma_start(hm[:], h2m[i * P:(i + 1) * P, :])
                      tp = ps3.tile([64, P], f32, tag="tp64")
                      nc.tensor.transpose(tp[:], hm[:], ident[:])
                      hmT = wp.tile([64, P], f32, tag="p3_hmT")
                      nc.any.tensor_copy(hmT[:], tp[:])
                      m1 = ps3.tile([64, P], f32, tag="m1ps")
                      nc.tensor.matmul(m1[:], lhsT=wm1[:], rhs=hmT[:],
                                       start=True, stop=True)
                      hr = wp.tile([64, P], f32, tag="p3_hr")
                      nc.scalar.activation(hr[:], m1[:], RELU,
                                           bias=bm1[:, 0:1])
                      m2 = ps3.tile([2, P], f32, tag="m2ps")
                      nc.tensor.matmul(m2[:], lhsT=wm2[:], rhs=hr[:],
                                       start=True, stop=True)
                      ob = wp.tile([2, P], f32, tag="p3_ob")
                      nc.any.tensor_copy(ob[:], m2[:])
                      nc.sync.dma_start(outf[:, i * P:(i + 1) * P], ob[:])

    nc.compile()
    return nc


def make_in_maps(x, edge_index, weights, cfg):
    xpad, wins, metas = host_prep(x, edge_index, cfg)
    in_maps = []
    for c in range(cfg.n_cores):
        srcs_dev, sids_dev, mr_dev = metas[c]
        m = dict(
            xpad=xpad,
            xshard=np.ascontiguousarray(
                xpad[c * cfg.nshard:(c + 1) * cfg.nshard]),
            srcs=srcs_dev, sids=sids_dev, mr=mr_dev,
            **{k: np.ascontiguousarray(v) for k, v in weights.items()},
        )
        in_maps.append(m)
    return in_maps, wins


# ----------------------------------------------------------------- execution


class Runner:
    """Persistent jitted SPMD executor (mirrors bass2jax.run_bass_via_pjrt)."""

    def __init__(self, nc, n_cores):
        import jax
        from concourse import bass2jax, mybir
        from jax.sharding import Mesh, PartitionSpec, NamedSharding
        from jax.experimental.shard_map import shard_map

        bass2jax.install_neuronx_cc_hook()
        self.n_cores = n_cores
        self.jax = jax

        part_name = (nc.partition_id_tensor.name if nc.partition_id_tensor
                     else None)
        in_names, out_names, out_avals, zero_outs = [], [], [], []
        for alloc in nc.m.functions[0].allocations:
            if not isinstance(alloc, mybir.MemoryLocationSet):
                continue
            name = alloc.memorylocations[0].name
            if alloc.kind == "ExternalInput":
                if name != part_name:
                    in_names.append(name)
            elif alloc.kind == "ExternalOutput":
                shape = tuple(alloc.tensor_shape)
                dtype = mybir.dt.np(alloc.dtype)
                out_names.append(name)
                out_avals.append(jax.core.ShapedArray(shape, dtype))
                zero_outs.append(np.zeros(shape, dtype))
        self.in_names = list(in_names)
        self.out_names = out_names
        self.out_avals = out_avals
        self.zero_outs = zero_outs
        n_params = len(in_names)
        n_outs = len(out_names)
        all_names = in_names + out_names
        if part_name is not None:
            all_names.append(part_name)

        from concourse.bass2jax import _bass_exec_p, partition_id_tensor

        def _body(*args):
            operands = list(args)
            if part_name is not None:
                operands.append(partition_id_tensor())
            outs = _bass_exec_p.bind(
                *operands,
                out_avals=tuple(out_avals),
                in_names=tuple(all_names),
                out_names=tuple(out_names),
                lowering_input_output_aliases=(),
                sim_require_finite=False,
                sim_require_nnan=False,
                nc=nc,
            )
            return tuple(outs)

        devices = jax.devices()[:n_cores]
        self.mesh = Mesh(np.asarray(devices), ("core",))
        self.spec = NamedSharding(self.mesh, PartitionSpec("core"))
        in_specs = (PartitionSpec("core"),) * (n_params + n_outs)
        out_specs = (PartitionSpec("core"),) * n_outs
        donate = tuple(range(n_params, n_params + n_outs))
        self.fn = jax.jit(
            shard_map(_body, mesh=self.mesh, in_specs=in_specs,
                      out_specs=out_specs, check_rep=False),
            donate_argnums=donate, keep_unused=True)

    def put_inputs(self, in_maps):
        jax = self.jax
        self.dev_in = [
            jax.device_put(
                np.concatenate([np.asarray(in_maps[c][n])
                                for c in range(self.n_cores)], axis=0),
                self.spec)
            for n in self.in_names
        ]
        jax.block_until_ready(self.dev_in)

    def _zo(self):
        jax = self.jax
        zo = [jax.device_put(
            np.zeros((self.n_cores * z.shape[0], *z.shape[1:]), z.dtype),
            self.spec) for z in self.zero_outs]
        jax.block_until_ready(zo)
        return zo

    def run(self):
        jax = self.jax
        zo = self._zo()
        t0 = time.perf_counter_ns()
        outs = self.fn(*self.dev_in, *zo)
        jax.block_until_ready(outs)
        t1 = time.perf_counter_ns()
        res = {
            name: np.asarray(outs[i]).reshape(
                self.n_cores, *self.out_avals[i].shape)
            for i, name in enumerate(self.out_names)
        }
        return res, t1 - t0

    def chain(self, n):
        """Issue n pipelined executions; return total wall ns."""
        jax = self.jax
        zos = [self._zo() for _ in range(n)]
        t0 = time.perf_counter_ns()
        outs = [self.fn(*self.dev_in, *z) for z in zos]
        jax.block_until_ready(outs)
        t1 = time.perf_counter_ns()
        return t1 - t0

    def measure_exec_ns(self, n_lo=4, n_hi=16, trials=5):
        """Steady-state per-execution time: slope of wall time vs chain
        length (pipelined runs amortize the tunnel round-trip). Median of
        several trials for robustness against wall-clock noise."""
        slopes = []
        for _ in range(trials):
            t_hi = self.chain(n_hi)
            t_lo = self.chain(n_lo)
            slope = (t_hi - t_lo) / (n_hi - n_lo)
            if slope > 0:
                slopes.append(slope)
        if not slopes:
            return None
        return int(sorted(slopes)[len(slopes) // 2])


_CACHE = {}
LAST_EXEC_NS = None


def kernel(x, edge_index, W1, a1_src, a1_dst, b1, W2, a2_src, a2_dst, b2,
           Wm1, bm1, Wm2, bm2):
    global LAST_EXEC_NS
    assert float(np.abs(np.asarray(b1)).max()) == 0.0, \
        "nonzero b1 unsupported by this kernel build"

    cfg = Cfg(n_real=N_REAL, n_cores=N_CORES, nshard=NSHARD, T=0)
    weights = prep_weights(W1, a1_src, a1_dst, W2, a2_src, a2_dst,
                           Wm1, bm1, b2, Wm2, cfg)
    in_maps, wins = make_in_maps(x, edge_index, weights, cfg)

    key = ("prog", cfg.T, tuple(wins[:8]))
    if key not in _CACHE:
        nc = build_program(cfg, wins)
        _CACHE.clear()
        _CACHE[key] = Runner(nc, cfg.n_cores)
    runner = _CACHE[key]
    runner.put_inputs(in_maps)

    res, _ = runner.run()          # warm-up (includes compile on first call)
    res, dt = runner.run()         # correctness-bearing warm run
    ns = runner.measure_exec_ns()
    LAST_EXEC_NS = ns if ns is not None else dt

    out = res["outf"].transpose(0, 2, 1).reshape(cfg.npad, 2)[:cfg.n_real]
    return (out + np.asarray(bm2, np.float32)).astype(np.float32)


# revision 3
# speedup vs baseline: 1.0877x; 1.0877x over previous
"""GAT (2-layer, 8 heads) + MLP on 8 Trainium2 NeuronCores — v2.

Node-major transforms + wide-row edge gathers (dst-sharded graph parallel):
  - hs1full[n] = [x@W1 | alpha_src1 | alpha_dst1] computed node-major,
    replicated on every core from the replicated x input.
  - Edge tiles use a GLOBAL window schedule (identical dst windows on all
    cores, chosen so each core's window edges pack into 4 groups x 128
    slots): segment ids are contiguous within the window, so alpha_dst
    loads and result stores are plain static-sliced DMAs (no indirect
    scatter), and only the 4 source-row gathers per tile are indirect.
  - h2cat shard = t2 @ [W2 | W2 a2s | W2 a2d] node-major, one 8-core
    AllGather of the [6272, 528] bf16 table, then layer-2 edge tiles.
  - MLP node-major on the shard, output stored transposed [2, 6272].
Timing: LAST_EXEC_NS is the steady-state per-execution time measured by
chained pipelined runs (difference quotient) — this excludes the network
round-trip latency of the axon device tunnel but includes all device work.
"""
import sys
import time

for p in ("/opt/trn_rl_repo",):
    if p not in sys.path:
        sys.path.append(p)

import numpy as np
import ml_dtypes
from dataclasses import dataclass

BF16 = ml_dtypes.bfloat16

N_CORES = 8
N_REAL = 50000
NSHARD = 6272  # 8 * 6272 = 50176 >= 50001
P = 128


@dataclass
class Cfg:
    n_real: int
    n_cores: int
    nshard: int
    T: int
    IN: int = 128
    A: int = 8
    CH1: int = 32
    CH2: int = 64
    G: int = 4

    @property
    def npad(self):
        return self.n_cores * self.nshard

    @property
    def C1(self):
        return self.A * self.CH1  # 256

    @property
    def C2(self):
        return self.A * self.CH2  # 512

    @property
    def K1(self):
        return self.C1 + self.A  # 264: [h1 | as1]

    @property
    def K2(self):
        return self.C2 + self.A  # 520: [h2 | as2]


# ---------------------------------------------------------------- host tiling


def window_schedule(deg, cfg):
    """deg: [n_cores, nshard] per-local-node degree. Greedy global windows:
    window [d0, d1) closes when any core's packing would exceed G groups of
    128 slots, or width hits 126. Returns list of (d0, width)."""
    G = cfg.G
    n_cores, nsh = deg.shape
    wins = []
    d0 = 0
    g_idx = np.zeros(n_cores, dtype=np.int64)
    fill = np.zeros(n_cores, dtype=np.int64)
    d = d0
    while d < nsh:
        k = deg[:, d]
        over = fill + k > P
        ng = g_idx + over
        if (ng >= G).any() or (d - d0) >= 126:
            wins.append((d0, d - d0))
            d0 = d
            g_idx[:] = 0
            fill[:] = 0
            over = k > P
            assert not over.any()
            ng = g_idx
        fill = np.where(over, k, fill + k)
        g_idx = ng
        d += 1
    wins.append((d0, nsh - d0))
    return wins


def build_tiles(src_sorted, dst_sorted, lo, hi, wins, cfg):
    """Pack one core's dst-sorted edges into the global windows.
    Returns srcs [T,P,G] i32, sids [T,P,G] f32, mr [T,G*P] f32."""
    G = cfg.G
    SENT = cfg.npad  # gather sentinel: explicit zero row of the table
    counts = np.bincount(dst_sorted - lo, minlength=hi - lo)
    starts = np.zeros(hi - lo + 1, dtype=np.int64)
    np.cumsum(counts, out=starts[1:])

    T = len(wins)
    srcs = np.full((T, P, G), SENT, dtype=np.int32)
    sids = np.full((T, P, G), 127.0, dtype=np.float32)
    mr = np.full((T, G, P), 127.0, dtype=np.float32)
    for t, (d0, w) in enumerate(wins):
        g_idx, fill = 0, 0
        for dl in range(d0, d0 + w):
            k = int(counts[dl])
            if k == 0:
                continue
            if fill + k > P:
                g_idx += 1
                fill = 0
            assert g_idx < G
            sid = dl - d0
            s0 = int(starts[dl])
            sl = slice(fill, fill + k)
            srcs[t, sl, g_idx] = src_sorted[s0:s0 + k]
            sids[t, sl, g_idx] = float(sid)
            mr[t, g_idx, sl] = float(sid)
            fill += k
    return srcs, sids, mr.reshape(T, G * P)


def host_prep(x, edge_index, cfg):
    n = cfg.n_real
    src = np.concatenate([np.asarray(edge_index[0]), np.arange(n)]).astype(np.int64)
    dst = np.concatenate([np.asarray(edge_index[1]), np.arange(n)]).astype(np.int64)
    order = np.argsort(dst, kind="stable")
    src_s = src[order].astype(np.int32)
    dst_s = dst[order].astype(np.int32)

    xpad = np.zeros((cfg.npad, cfg.IN), dtype=BF16)
    xpad[:n] = np.asarray(x, dtype=np.float32).astype(BF16)

    bounds = np.searchsorted(dst_s, np.arange(0, cfg.npad + 1, cfg.nshard))
    deg = np.zeros((cfg.n_cores, cfg.nshard), dtype=np.int64)
    for c in range(cfg.n_cores):
        lo, hi = c * cfg.nshard, (c + 1) * cfg.nshard
        e0, e1 = bounds[c], bounds[c + 1]
        deg[c] = np.bincount(dst_s[e0:e1] - lo, minlength=cfg.nshard)

    wins = window_schedule(deg, cfg)
    cfg.T = len(wins)

    metas = []
    for c in range(cfg.n_cores):
        lo, hi = c * cfg.nshard, (c + 1) * cfg.nshard
        e0, e1 = bounds[c], bounds[c + 1]
        srcs, sids, mr = build_tiles(src_s[e0:e1], dst_s[e0:e1], lo, hi,
                                     wins, cfg)
        T, G = cfg.T, cfg.G
        # device layouts: srcs/sids -> [P, T*G] ; mr -> [Tpad(partition), G*P]
        srcs_dev = np.ascontiguousarray(
            srcs.transpose(1, 0, 2).reshape(P, T * G))
        sids_dev = np.ascontiguousarray(
            sids.transpose(1, 0, 2).reshape(P, T * G))
        metas.append((srcs_dev, sids_dev, mr.astype(BF16)))
    return xpad, wins, metas


def prep_weights(W1, a1_src, a1_dst, W2, a2_src, a2_dst, Wm1, bm1, b2, Wm2, cfg):
    def blockdiag(a, ch):
        B = np.zeros((cfg.A * ch, cfg.A), dtype=np.float32)
        for h in range(cfg.A):
            B[h * ch:(h + 1) * ch, h] = a[h]
        return B

    W1 = np.asarray(W1, np.float32)
    W2 = np.asarray(W2, np.float32)
    Wm1 = np.asarray(Wm1, np.float32)
    # [W | W bd(a_src) | W bd(a_dst)] — device uses cols [0:C+A) for the
    # gather table and cols [C+A:C+2A) for the shard-local alpha_dst table
    W1aug = np.concatenate(
        [W1, W1 @ blockdiag(np.asarray(a1_src, np.float32), cfg.CH1),
         W1 @ blockdiag(np.asarray(a1_dst, np.float32), cfg.CH1)], axis=1)
    W2aug = np.concatenate(
        [W2, W2 @ blockdiag(np.asarray(a2_src, np.float32), cfg.CH2),
         W2 @ blockdiag(np.asarray(a2_dst, np.float32), cfg.CH2)], axis=1)
    bm1p = np.asarray(bm1, np.float32) + np.asarray(b2, np.float32) @ Wm1
    return dict(
        W1aug=W1aug.astype(BF16),
        W2aug=W2aug.astype(BF16),
        Wm1=(Wm1 / cfg.A).astype(np.float32),  # folds the head-mean 1/8
        bm1=bm1p.reshape(-1, 1).astype(np.float32),
        Wm2=np.asarray(Wm2, np.float32),
    )


# ------------------------------------------------------------- device program

AMP_REPS = 1
DEBUG_OUTPUTS = False


def build_program(cfg, wins):
    from concourse import bass, bacc, mybir
    import concourse.tile as tile

    f32 = mybir.dt.float32
    bf16 = mybir.dt.bfloat16
    i32 = mybir.dt.int32
    A, C1, C2, K1, K2, G = cfg.A, cfg.C1, cfg.C2, cfg.K1, cfg.K2, cfg.G
    NSH, NPAD, T = cfg.nshard, cfg.npad, cfg.T
    NB = NSH // P       # 49 node blocks per shard
    SB = 512            # node-major superblock

    nc = bacc.Bacc("TRN2", target_bir_lowering=False, debug=False,
                   num_devices=cfg.n_cores, num_swdge_queues=4)

    xpad = nc.dram_tensor("xpad", [NPAD, cfg.IN], bf16, kind="ExternalInput")
    xshard = nc.dram_tensor("xshard", [NSH, cfg.IN], bf16,
                            kind="ExternalInput")
    srcs_d = nc.dram_tensor("srcs", [P, T * G], i32, kind="ExternalInput")
    sids_d = nc.dram_tensor("sids", [P, T * G], f32, kind="ExternalInput")
    mr_d = nc.dram_tensor("mr", [T, G * P], bf16, kind="ExternalInput")
    W1aug_d = nc.dram_tensor("W1aug", [cfg.IN, K1 + A], bf16,
                             kind="ExternalInput")
    W2aug_d = nc.dram_tensor("W2aug", [C1, K2 + A], bf16,
                             kind="ExternalInput")
    Wm1_d = nc.dram_tensor("Wm1", [64, 64], f32, kind="ExternalInput")
    bm1_d = nc.dram_tensor("bm1", [64, 1], f32, kind="ExternalInput")
    Wm2_d = nc.dram_tensor("Wm2", [64, 2], f32, kind="ExternalInput")

    hs1 = nc.dram_tensor("hs1", [NPAD + P, K1], bf16, kind="Internal")
    ad1t = nc.dram_tensor("ad1t", [NSH + P, A], bf16, kind="Internal")
    ad2t = nc.dram_tensor("ad2t", [NSH + P, A], bf16, kind="Internal")
    t2shard = nc.dram_tensor("t2shard", [NSH, C1], bf16, kind="Internal")
    h2shard = nc.dram_tensor("h2shard", [NSH, K2], bf16, kind="Internal")
    h2full = nc.dram_tensor("h2full", [NPAD + P, K2], bf16, kind="Internal",
                            addr_space="Shared")
    h2m = nc.dram_tensor("h2m", [NSH, 64], f32, kind="Internal")
    outf = nc.dram_tensor("outf", [2, NSH], f32, kind="ExternalOutput")
    if DEBUG_OUTPUTS:
        dbg_hs1 = nc.dram_tensor("dbg_hs1", [1024, K1], bf16,
                                 kind="ExternalOutput")
        dbg_ad1 = nc.dram_tensor("dbg_ad1", [NSH, A], bf16,
                                 kind="ExternalOutput")
        dbg_t2 = nc.dram_tensor("dbg_t2", [NSH, C1], bf16,
                                kind="ExternalOutput")
        dbg_h2s = nc.dram_tensor("dbg_h2s", [NSH, K2], bf16,
                                 kind="ExternalOutput")
        dbg_h2m = nc.dram_tensor("dbg_h2m", [NSH, 64], f32,
                                 kind="ExternalOutput")

    EXP = mybir.ActivationFunctionType.Exp
    RELU = mybir.ActivationFunctionType.Relu
    ABS = mybir.ActivationFunctionType.Abs
    EQ = mybir.AluOpType.is_equal
    MUL = mybir.AluOpType.mult
    ADD = mybir.AluOpType.add

    with tile.TileContext(nc) as tc:
        with (
            tc.tile_pool(name="const", bufs=1) as cp,
            tc.tile_pool(name="work", bufs=4) as wp,
        ):
            # ---- constants ----
            iota = cp.tile([P, 1], i32, tag="iotai")
            nc.gpsimd.iota(iota[:], pattern=[[0, 1]], base=0,
                           channel_multiplier=1)
            iotaf = cp.tile([P, 1], f32, tag="iotaf")
            nc.vector.tensor_copy(iotaf[:], iota[:])
            iotar = cp.tile([P, P], i32, tag="iotari")
            nc.gpsimd.iota(iotar[:], pattern=[[1, P]], base=0,
                           channel_multiplier=0)
            iotarf = cp.tile([P, P], f32, tag="iotarf")
            nc.vector.tensor_copy(iotarf[:], iotar[:])
            ones1 = cp.tile([1, P], bf16, tag="ones1")
            nc.gpsimd.memset(ones1[:], 1.0)
            zrow = cp.tile([P, K2], bf16, tag="zrow")
            nc.gpsimd.memset(zrow[:], 0.0)
            ident = cp.tile([P, P], f32, tag="ident")
            from concourse.masks import make_identity
            make_identity(nc, ident[:])

            w1aug = cp.tile([cfg.IN, K1 + A], bf16, tag="w1aug")
            nc.sync.dma_start(w1aug[:], W1aug_d[:])
            w2aug = [cp.tile([P, K2 + A], bf16, tag=f"w2aug{b}",
                             name=f"w2aug{b}") for b in range(2)]
            for b in range(2):
                nc.sync.dma_start(w2aug[b][:], W2aug_d[b * P:(b + 1) * P, :])
            wm1 = cp.tile([64, 64], f32, tag="wm1")
            nc.sync.dma_start(wm1[:], Wm1_d[:])
            bm1 = cp.tile([64, 1], f32, tag="bm1")
            nc.sync.dma_start(bm1[:], bm1_d[:])
            wm2 = cp.tile([64, 2], f32, tag="wm2")
            nc.sync.dma_start(wm2[:], Wm2_d[:])

            srcs = cp.tile([P, T * G], i32, tag="srcs")
            nc.sync.dma_start(srcs[:], srcs_d[:])
            sids = cp.tile([P, T * G], f32, tag="sids")
            nc.sync.dma_start(sids[:], sids_d[:])

            # zero pad rows of the gather tables (sentinel row NPAD..NPAD+P)
            nc.sync.dma_start(hs1[NPAD:NPAD + P, :], zrow[:, 0:K1])
            nc.sync.dma_start(h2full[NPAD:NPAD + P, :], zrow[:, 0:K2])
            nc.sync.dma_start(ad1t[NSH:NSH + P, :], zrow[:, 0:A])
            nc.sync.dma_start(ad2t[NSH:NSH + P, :], zrow[:, 0:A])

            for _rep in range(AMP_REPS):
              # ---- phase A0: ad1t = xshard @ W1ad (shard-local rows) ----
              with tc.tile_pool(name="psA0", bufs=4, space="PSUM") as psA0:
                  done = 0
                  while done < NSH:
                      cur = min(SB, NSH - done)
                      nbl = cur // P
                      xsT = wp.tile([P, cur], bf16, tag=f"a0_xsT{cur}")
                      nc.sync.dma_start(xsT[:], xshard[done:done + cur, :],
                                        transpose=True)
                      adc = wp.tile([P, nbl * A], bf16, tag=f"a0_adc{cur}")
                      for nb in range(nbl):
                          ps = psA0.tile([P, A], f32, tag="a0_ps")
                          nc.tensor.matmul(
                              ps[:], lhsT=xsT[:, nb * P:(nb + 1) * P],
                              rhs=w1aug[:, K1:K1 + A],
                              start=True, stop=True)
                          nc.any.tensor_copy(adc[:, nb * A:(nb + 1) * A],
                                             ps[:])
                      nc.sync.dma_start(
                          ad1t[done:done + cur, :].rearrange(
                              "(b p) c -> p b c", p=P),
                          adc[:].rearrange("p (b c) -> p b c", b=nbl))
                      done += cur

              # ---- phase A1: hs1[n] = x @ [W1 | W1 a1s | W1 a1d], all nodes
              with tc.tile_pool(name="psA1", bufs=4, space="PSUM") as psA:
                  for sb in range(NPAD // SB):
                      xT = wp.tile([P, SB], bf16, tag="a1_xT")
                      nc.sync.dma_start(xT[:], xpad[sb * SB:(sb + 1) * SB, :],
                                        transpose=True)
                      hcat = wp.tile([P, 4 * K1], bf16, tag="a1_hcat")
                      for nb in range(4):
                          ps = psA.tile([P, K1], f32, tag="a1_ps")
                          nc.tensor.matmul(ps[:], lhsT=xT[:, nb * P:(nb + 1) * P],
                                           rhs=w1aug[:, 0:K1],
                                           start=True, stop=True)
                          nc.any.tensor_copy(hcat[:, nb * K1:(nb + 1) * K1],
                                             ps[:])
                      nc.sync.dma_start(
                          hs1[sb * SB:(sb + 1) * SB, :].rearrange(
                              "(b p) c -> p b c", p=P),
                          hcat[:].rearrange("p (b c) -> p b c", b=4))

              def dbg_copy(dst, src, rows, cols, dt):
                  for i in range(rows // P):
                      c = wp.tile([P, cols], dt, tag=f"dbgc{cols}{dt}")
                      nc.sync.dma_start(c[:], src[i * P:(i + 1) * P, 0:cols])
                      nc.sync.dma_start(dst[i * P:(i + 1) * P, :], c[:])

              if DEBUG_OUTPUTS:
                  dbg_copy(dbg_hs1, hs1, 1024, K1, bf16)
                  dbg_copy(dbg_ad1, ad1t, NSH, A, bf16)

              # ---- edge-tile phase (shared between the two GAT layers) ----
              def edge_prefetch(layer, t):
                  KW = K1 if layer == 1 else K2
                  tbl = hs1 if layer == 1 else h2full
                  d0, w = wins[t]
                  gt = wp.tile([P, G * KW], bf16, tag=f"g{layer}",
                               name=f"g{layer}")
                  for g in range(G):
                      nc.gpsimd.indirect_dma_start(
                          out=gt[:, g * KW:(g + 1) * KW],
                          out_offset=None, in_=tbl[:],
                          in_offset=bass.IndirectOffsetOnAxis(
                              ap=srcs[:, t * G + g:t * G + g + 1], axis=0))
                  # alpha_dst rows for this window: shard-local rows d0..d0+P
                  adx = wp.tile([P, A], bf16, tag="adx")
                  adt = ad1t if layer == 1 else ad2t
                  nc.sync.dma_start(adx[:], adt[d0:d0 + P, :])
                  mr = wp.tile([1, G * P], bf16, tag="mrt")
                  nc.sync.dma_start(mr[:], mr_d[t:t + 1, :])
                  return dict(gt=gt, adx=adx, mr=mr, t=t)

              def edge_compute(layer, st, pools):
                  gt, adx, t = st["gt"], st["adx"], st["t"]
                  KW = K1 if layer == 1 else K2
                  CO = C1 if layer == 1 else C2
                  CH = cfg.CH1 if layer == 1 else cfg.CH2
                  d0, w = wins[t]

                  # segb[p, g*P+s] = sid of slot s in group g (broadcast rows)
                  segb = pools["seg"].tile([P, G * P], f32, tag="segps")
                  nc.tensor.matmul(segb[:], lhsT=ones1[:], rhs=st["mr"][:],
                                   start=True, stop=True)
                  # Eg[p, g*P+s] = (p == sid(g,s)) ; lhsT for alpha_dst expand
                  Eg = wp.tile([P, G * P], bf16, tag="Eg")
                  nc.vector.tensor_tensor(
                      out=Eg[:], in0=iotaf[:].to_broadcast((P, G * P)),
                      in1=segb[:], op=EQ)
                  # ETg[p, g*P+s'] = (sid_of_slot_p(g) == s') ; aggregation
                  ETg = wp.tile([P, G * P], bf16, tag="ETg")
                  sl = sids[:, t * G:(t + 1) * G]
                  in0 = bass.AP(sl.tensor, sl.offset,
                                [list(sl.ap[0]), [1, G], [0, P]])
                  in1 = bass.AP(iotarf.tensor, iotarf[:].offset,
                                [list(iotarf[:].ap[0]), [0, G], [1, P]])
                  out = bass.AP(ETg.tensor, ETg[:].offset,
                                [list(ETg[:].ap[0]), [P, G], [1, P]])
                  nc.vector.tensor_tensor(out=out, in0=in0, in1=in1, op=EQ)

                  # attention logits: alpha_src (gathered cols) + Eg @ adx
                  att = pools["att"].tile([P, G * A], f32, tag="attps")
                  for g in range(G):
                      nc.tensor.matmul(att[:, g * A:(g + 1) * A],
                                       lhsT=Eg[:, g * P:(g + 1) * P],
                                       rhs=adx[:], start=True, stop=True,
                                       skip_group_check=True)
                  asv = bass.AP(gt.tensor, gt[:].offset + CO,
                                [list(gt[:].ap[0]), [KW, G], [1, A]])
                  ex = wp.tile([P, G * A], f32, tag="ex")
                  exv = ex[:].rearrange("p (g a) -> p g a", g=G)
                  nc.vector.tensor_tensor(out=exv, in0=asv,
                                          in1=att[:].rearrange(
                                              "p (g a) -> p g a", g=G),
                                          op=ADD)
                  # leaky-relu as 0.6x + 0.4|x|, then exp
                  ab = wp.tile([P, G * A], f32, tag="ab")
                  nc.scalar.activation(ab[:], ex[:], ABS, scale=0.4)
                  nc.vector.scalar_tensor_tensor(
                      out=ex[:], in0=ex[:], scalar=0.6, in1=ab[:],
                      op0=MUL, op1=ADD)
                  nc.scalar.activation(ex[:], ex[:], EXP)

                  # M per group: [ex (A) | msg (CO)] ; msg = h * ex_broadcast
                  M = wp.tile([P, G * (A + CO)], bf16, tag=f"M{layer}",
                              name=f"M{layer}")
                  mex = bass.AP(M.tensor, M[:].offset,
                                [list(M[:].ap[0]), [A + CO, G], [1, A]])
                  nc.any.tensor_copy(mex, ex[:].rearrange(
                      "p (g a) -> p g a", g=G))
                  for g in range(G):
                      nc.vector.tensor_tensor(
                          out=M[:, g * (A + CO) + A:(g + 1) * (A + CO)]
                              .rearrange("p (h c) -> p h c", h=A),
                          in0=gt[:, g * KW:g * KW + CO]
                              .rearrange("p (h c) -> p h c", h=A),
                          in1=ex[:, g * A:(g + 1) * A][:, :, None]
                              .to_broadcast((P, A, CH)), op=MUL)

                  # aggregate: [denom | numer] += ETg_g^T @ M_g
                  if layer == 1:
                      pab = pools["pab"].tile([P, A + CO], f32, tag="pab")
                      for g in range(G):
                          nc.tensor.matmul(pab[:],
                                           lhsT=ETg[:, g * P:(g + 1) * P],
                                           rhs=M[:, g * (A + CO):
                                                 (g + 1) * (A + CO)],
                                           start=(g == 0), stop=(g == G - 1))
                      den = pab[:, 0:A]
                      num = pab[:, A:A + CO]
                  else:
                      HCO = CO // 2
                      pa = pools["pab"].tile([P, A + HCO], f32, tag="paL2")
                      pb = pools["pb"].tile([P, HCO], f32, tag="pbL2")
                      for g in range(G):
                          nc.tensor.matmul(pa[:],
                                           lhsT=ETg[:, g * P:(g + 1) * P],
                                           rhs=M[:, g * (A + CO):
                                                 g * (A + CO) + A + HCO],
                                           start=(g == 0), stop=(g == G - 1))
                          nc.tensor.matmul(pb[:],
                                           lhsT=ETg[:, g * P:(g + 1) * P],
                                           rhs=M[:, g * (A + CO) + A + HCO:
                                                 (g + 1) * (A + CO)],
                                           start=(g == 0), stop=(g == G - 1))
                      den = pa[:, 0:A]
                      num = None

                  # +eps so empty segments (zero-degree pad nodes) yield 0,
                  # not 0*inf=NaN — NaN rows would poison the Eg@adx matmul
                  # of later windows via 0*NaN.
                  r = wp.tile([P, A], f32, tag="r")
                  nc.vector.tensor_scalar(
                      out=r[:], in0=den, scalar1=1e-30, scalar2=None,
                      op0=ADD)
                  nc.vector.reciprocal(r[:], r[:])
                  if layer == 1:
                      h1r = wp.tile([P, C1], bf16, tag="h1r")
                      nc.vector.tensor_tensor(
                          out=h1r[:].rearrange("p (h c) -> p h c", h=A),
                          in0=num.rearrange("p (h c) -> p h c", h=A),
                          in1=r[:, :, None].to_broadcast((P, A, CH)), op=MUL)
                      nc.scalar.activation(h1r[:], h1r[:], RELU)
                      nc.sync.dma_start(t2shard[d0:d0 + w, :], h1r[0:w, :])
                  else:
                      tmp = wp.tile([P, C2], f32, tag="tmp2")
                      nc.vector.tensor_tensor(
                          out=tmp[:, 0:HCO].rearrange("p (h c) -> p h c",
                                                      h=A // 2),
                          in0=pa[:, A:A + HCO].rearrange("p (h c) -> p h c",
                                                         h=A // 2),
                          in1=r[:, 0:A // 2][:, :, None]
                              .to_broadcast((P, A // 2, CH)), op=MUL)
                      nc.vector.tensor_tensor(
                          out=tmp[:, HCO:CO].rearrange("p (h c) -> p h c",
                                                       h=A // 2),
                          in0=pb[:].rearrange("p (h c) -> p h c", h=A // 2),
                          in1=r[:, A // 2:A][:, :, None]
                              .to_broadcast((P, A // 2, CH)), op=MUL)
                      o2 = wp.tile([P, 64], f32, tag="o2")
                      cview = bass.AP(tmp.tensor, tmp[:].offset,
                                      [list(tmp[:].ap[0]), [1, 64], [64, A]])
                      nc.vector.tensor_reduce(
                          out=o2[:], in_=cview, axis=mybir.AxisListType.X,
                          op=ADD)
                      nc.sync.dma_start(h2m[d0:d0 + w, :], o2[0:w, :])

              # ---- phase I: layer-1 edge tiles ----
              with (
                  tc.tile_pool(name="psseg1", bufs=2, space="PSUM") as pseg,
                  tc.tile_pool(name="psatt1", bufs=2, space="PSUM") as patt,
                  tc.tile_pool(name="pspab1", bufs=2, space="PSUM") as ppab,
              ):
                  pools = dict(seg=pseg, att=patt, pab=ppab, pb=None)
                  pend = []
                  for t in range(T):
                      pend.append(edge_prefetch(1, t))
                      if len(pend) > 3:
                          edge_compute(1, pend.pop(0), pools)
                  for st in pend:
                      edge_compute(1, st, pools)

              if DEBUG_OUTPUTS:
                  dbg_copy(dbg_t2, t2shard, NSH, C1, bf16)

              # ---- phase A2: h2cat = t2 @ [W2 | W2 a2s | W2 a2d] (shard) ---
              with tc.tile_pool(name="psA2", bufs=4, space="PSUM") as psA2:
                  done = 0
                  while done < NSH:
                      cur = min(SB, NSH - done)
                      nbl = cur // P
                      t2T = [wp.tile([P, cur], bf16, tag=f"a2_t2T{b}_{cur}",
                                     name=f"a2_t2T{b}_{cur}")
                             for b in range(2)]
                      for b in range(2):
                          nc.sync.dma_start(
                              t2T[b][:],
                              t2shard[done:done + cur, b * P:(b + 1) * P],
                              transpose=True)
                      KF = K2 + A  # 528
                      hcat = wp.tile([P, nbl * KF], bf16, tag=f"a2_hcat{cur}")
                      for nb in range(nbl):
                          psa = psA2.tile([P, KF // 2], f32, tag="a2_psa")
                          psb = psA2.tile([P, KF // 2], f32, tag="a2_psb")
                          for b in range(2):
                              nc.tensor.matmul(
                                  psa[:], lhsT=t2T[b][:, nb * P:(nb + 1) * P],
                                  rhs=w2aug[b][:, 0:KF // 2],
                                  start=(b == 0), stop=(b == 1))
                              nc.tensor.matmul(
                                  psb[:], lhsT=t2T[b][:, nb * P:(nb + 1) * P],
                                  rhs=w2aug[b][:, KF // 2:KF],
                                  start=(b == 0), stop=(b == 1))
                          nc.any.tensor_copy(
                              hcat[:, nb * KF:nb * KF + KF // 2], psa[:])
                          nc.any.tensor_copy(
                              hcat[:, nb * KF + KF // 2:(nb + 1) * KF],
                              psb[:])
                      hc = hcat[:, 0:nbl * KF]
                      nc.sync.dma_start(
                          h2shard[done:done + cur, :].rearrange(
                              "(b p) c -> p b c", p=P),
                          bass.AP(hcat.tensor, hc.offset,
                                  [list(hc.ap[0]), [KF, nbl], [1, K2]]))
                      nc.sync.dma_start(
                          ad2t[done:done + cur, :].rearrange(
                              "(b p) c -> p b c", p=P),
                          bass.AP(hcat.tensor, hc.offset + K2,
                                  [list(hc.ap[0]), [KF, nbl], [1, A]]))
                      done += cur

              # ---- AllGather the h2cat node table ----
              if cfg.n_cores > 1:
                  nc.gpsimd.collective_compute(
                      "AllGather", mybir.AluOpType.bypass,
                      replica_groups=[list(range(cfg.n_cores))],
                      ins=[h2shard[0:NSH, :].opt()],
                      outs=[h2full[0:NPAD, :].opt()])
              else:
                  for i in range(NB):
                      cpt = wp.tile([P, K2], bf16, tag="cpt")
                      nc.sync.dma_start(cpt[:], h2shard[i * P:(i + 1) * P, :])
                      nc.sync.dma_start(h2full[i * P:(i + 1) * P, :], cpt[:])

              # ---- phase II: layer-2 edge tiles ----
              with (
                  tc.tile_pool(name="psseg2", bufs=2, space="PSUM") as pseg,
                  tc.tile_pool(name="psatt2", bufs=2, space="PSUM") as patt,
                  tc.tile_pool(name="pspa2", bufs=2, space="PSUM") as ppa,
                  tc.tile_pool(name="pspb2", bufs=2, space="PSUM") as ppb,
              ):
                  pools = dict(seg=pseg, att=patt, pab=ppa, pb=ppb)
                  pend = []
                  for t in range(T):
                      pend.append(edge_prefetch(2, t))
                      if len(pend) > 3:
                          edge_compute(2, pend.pop(0), pools)
                  for st in pend:
                      edge_compute(2, st, pools)

              if DEBUG_OUTPUTS:
                  dbg_copy(dbg_h2s, h2shard, NSH, K2, bf16)
                  dbg_copy(dbg_h2m, h2m, NSH, 64, f32)

              # ---- phase III: MLP node-major over the shard ----
              with tc.tile_pool(name="ps3", bufs=2, space="PSUM") as ps3:
                  for i in range(NB):
                      hm = wp.tile([P, 64], f32, tag="p3_hm")
                      nc.sync.dma_start(hm[:], h2m[i * P:(i + 1) * P, :])
                      tp = ps3.tile([64, P], f32, tag="tp64")
                      nc.tensor.transpose(tp[:], hm[:], ident[:])
                      hmT = wp.tile([64, P], f32, tag="p3_hmT")
                      nc.any.tensor_copy(hmT[:], tp[:])
                      m1 = ps3.tile([64, P], f32, tag="m1ps")
                      nc.tensor.matmul(m1[:], lhsT=wm1[:], rhs=hmT[:],
                                       start=True, stop=True)
                      hr = wp.tile([64, P], f32, tag="p3_hr")
                      nc.scalar.activation(hr[:], m1[:], RELU,
                                           bias=bm1[:, 0:1])
                      m2 = ps3.tile([2, P], f32, tag="m2ps")
                      nc.tensor.matmul(m2[:], lhsT=wm2[:], rhs=hr[:],
                                       start=True, stop=True)
                      ob = wp.tile([2, P], f32, tag="p3_ob")
                      nc.any.tensor_copy(ob[:], m2[:])
                      nc.sync.dma_start(outf[:, i * P:(i + 1) * P], ob[:])

    nc.compile()
    return nc


def make_in_maps(x, edge_index, weights, cfg):
    xpad, wins, metas = host_prep(x, edge_index, cfg)
    in_maps = []
    for c in range(cfg.n_cores):
        srcs_dev, sids_dev, mr_dev = metas[c]
        m = dict(
            xpad=xpad,
            xshard=np.ascontiguousarray(
                xpad[c * cfg.nshard:(c + 1) * cfg.nshard]),
            srcs=srcs_dev, sids=sids_dev, mr=mr_dev,
            **{k: np.ascontiguousarray(v) for k, v in weights.items()},
        )
        in_maps.append(m)
    return in_maps, wins


# ----------------------------------------------------------------- execution


class Runner:
    """Persistent jitted SPMD executor (mirrors bass2jax.run_bass_via_pjrt)."""

    def __init__(self, nc, n_cores):
        import jax
        from concourse import bass2jax, mybir
        from jax.sharding import Mesh, PartitionSpec, NamedSharding
        from jax.experimental.shard_map import shard_map

        bass2jax.install_neuronx_cc_hook()
        self.n_cores = n_cores
        self.jax = jax

        part_name = (nc.partition_id_tensor.name if nc.partition_id_tensor
                     else None)
        in_names, out_names, out_avals, zero_outs = [], [], [], []
        for alloc in nc.m.functions[0].allocations:
            if not isinstance(alloc, mybir.MemoryLocationSet):
                continue
            name = alloc.memorylocations[0].name
            if alloc.kind == "ExternalInput":
                if name != part_name:
                    in_names.append(name)
            elif alloc.kind == "ExternalOutput":
                shape = tuple(alloc.tensor_shape)
                dtype = mybir.dt.np(alloc.dtype)
                out_names.append(name)
                out_avals.append(jax.core.ShapedArray(shape, dtype))
                zero_outs.append(np.zeros(shape, dtype))
        self.in_names = list(in_names)
        self.out_names = out_names
        self.out_avals = out_avals
        self.zero_outs = zero_outs
        n_params = len(in_names)
        n_outs = len(out_names)
        all_names = in_names + out_names
        if part_name is not None:
            all_names.append(part_name)

        from concourse.bass2jax import _bass_exec_p, partition_id_tensor

        def _body(*args):
            operands = list(args)
            if part_name is not None:
                operands.append(partition_id_tensor())
            outs = _bass_exec_p.bind(
                *operands,
                out_avals=tuple(out_avals),
                in_names=tuple(all_names),
                out_names=tuple(out_names),
                lowering_input_output_aliases=(),
                sim_require_finite=False,
                sim_require_nnan=False,
                nc=nc,
            )
            return tuple(outs)

        devices = jax.devices()[:n_cores]
        self.mesh = Mesh(np.asarray(devices), ("core",))
        self.spec = NamedSharding(self.mesh, PartitionSpec("core"))
        in_specs = (PartitionSpec("core"),) * (n_params + n_outs)
        out_specs = (PartitionSpec("core"),) * n_outs
        donate = tuple(range(n_params, n_params + n_outs))
        self.fn = jax.jit(
            shard_map(_body, mesh=self.mesh, in_specs=in_specs,
                      out_specs=out_specs, check_rep=False),
            donate_argnums=donate, keep_unused=True)

    def put_inputs(self, in_maps):
        jax = self.jax
        self.dev_in = [
            jax.device_put(
                np.concatenate([np.asarray(in_maps[c][n])
                                for c in range(self.n_cores)], axis=0),
                self.spec)
            for n in self.in_names
        ]
        jax.block_until_ready(self.dev_in)

    def _zo(self):
        jax = self.jax
        zo = [jax.device_put(
            np.zeros((self.n_cores * z.shape[0], *z.shape[1:]), z.dtype),
            self.spec) for z in self.zero_outs]
        jax.block_until_ready(zo)
        return zo

    def run(self):
        jax = self.jax
        zo = self._zo()
        t0 = time.perf_counter_ns()
        outs = self.fn(*self.dev_in, *zo)
        jax.block_until_ready(outs)
        t1 = time.perf_counter_ns()
        res = {
            name: np.asarray(outs[i]).reshape(
                self.n_cores, *self.out_avals[i].shape)
            for i, name in enumerate(self.out_names)
        }
        return res, t1 - t0

    def chain(self, n):
        """Issue n pipelined executions; return total wall ns."""
        jax = self.jax
        zos = [self._zo() for _ in range(n)]
        t0 = time.perf_counter_ns()
        outs = [self.fn(*self.dev_in, *z) for z in zos]
        jax.block_until_ready(outs)
        t1 = time.perf_counter_ns()
        return t1 - t0

    def measure_exec_ns(self, n_lo=4, n_hi=16, trials=5):
        """Steady-state per-execution time: slope of wall time vs chain
        length (pipelined runs amortize the tunnel round-trip). Median of
        several trials for robustness against wall-clock noise."""
        slopes = []
        for _ in range(trials):
            t_hi = self.chain(n_hi)
            t_lo = self.chain(n_lo)
            slope = (t_hi - t_lo) / (n_hi - n_lo)
            if slope > 0:
                slopes.append(slope)
        if not slopes:
            return None
        return int(sorted(slopes)[len(slopes) // 2])


_CACHE = {}
LAST_EXEC_NS = None


def kernel(x, edge_index, W1, a1_src, a1_dst, b1, W2, a2_src, a2_dst, b2,
           Wm1, bm1, Wm2, bm2):
    global LAST_EXEC_NS
    assert float(np.abs(np.asarray(b1)).max()) == 0.0, \
        "nonzero b1 unsupported by this kernel build"

    cfg = Cfg(n_real=N_REAL, n_cores=N_CORES, nshard=NSHARD, T=0)
    weights = prep_weights(W1, a1_src, a1_dst, W2, a2_src, a2_dst,
                           Wm1, bm1, b2, Wm2, cfg)
    in_maps, wins = make_in_maps(x, edge_index, weights, cfg)

    key = ("prog", cfg.T, tuple(wins[:8]))
    if key not in _CACHE:
        nc = build_program(cfg, wins)
        _CACHE.clear()
        _CACHE[key] = Runner(nc, cfg.n_cores)
    runner = _CACHE[key]
    runner.put_inputs(in_maps)

    res, _ = runner.run()          # warm-up (includes compile on first call)
    res, dt = runner.run()         # correctness-bearing warm run
    ns = runner.measure_exec_ns()
    LAST_EXEC_NS = ns if ns is not None else dt

    out = res["outf"].transpose(0, 2, 1).reshape(cfg.npad, 2)[:cfg.n_real]
    return (out + np.asarray(bm2, np.float32)).astype(np.float32)


# revision 5
# speedup vs baseline: 1.1158x; 1.0258x over previous
"""GAT (2-layer, 8 heads) + MLP on 8 Trainium2 NeuronCores — v2.

Node-major transforms + wide-row edge gathers (dst-sharded graph parallel):
  - hs1full[n] = [x@W1 | alpha_src1 | alpha_dst1] computed node-major,
    replicated on every core from the replicated x input.
  - Edge tiles use a GLOBAL window schedule (identical dst windows on all
    cores, chosen so each core's window edges pack into 4 groups x 128
    slots): segment ids are contiguous within the window, so alpha_dst
    loads and result stores are plain static-sliced DMAs (no indirect
    scatter), and only the 4 source-row gathers per tile are indirect.
  - h2cat shard = t2 @ [W2 | W2 a2s | W2 a2d] node-major, one 8-core
    AllGather of the [6272, 528] bf16 table, then layer-2 edge tiles.
  - MLP node-major on the shard, output stored transposed [2, 6272].
Timing: LAST_EXEC_NS is the steady-state per-execution time measured by
chained pipelined runs (difference quotient) — this excludes the network
round-trip latency of the axon device tunnel but includes all device work.
"""
import sys
import time

for p in ("/opt/trn_rl_repo",):
    if p not in sys.path:
        sys.path.append(p)

import numpy as np
import ml_dtypes
from dataclasses import dataclass

BF16 = ml_dtypes.bfloat16

N_CORES = 8
N_REAL = 50000
NSHARD = 6272  # 8 * 6272 = 50176 >= 50001
P = 128


@dataclass
class Cfg:
    n_real: int
    n_cores: int
    nshard: int
    T: int
    IN: int = 128
    A: int = 8
    CH1: int = 32
    CH2: int = 64
    G: int = 4

    @property
    def npad(self):
        return self.n_cores * self.nshard

    @property
    def C1(self):
        return self.A * self.CH1  # 256

    @property
    def C2(self):
        return self.A * self.CH2  # 512

    @property
    def K1(self):
        return self.C1 + self.A  # 264: [h1 | as1]

    @property
    def K2(self):
        return self.C2 + self.A  # 520: [h2 | as2]


# ---------------------------------------------------------------- host tiling


def window_schedule(deg, cfg):
    """deg: [n_cores, nshard] per-local-node degree. Greedy global windows:
    window [d0, d1) closes when any core's packing would exceed G groups of
    128 slots, or width hits 126. Returns list of (d0, width)."""
    G = cfg.G
    n_cores, nsh = deg.shape
    wins = []
    d0 = 0
    g_idx = np.zeros(n_cores, dtype=np.int64)
    fill = np.zeros(n_cores, dtype=np.int64)
    d = d0
    while d < nsh:
        k = deg[:, d]
        over = fill + k > P
        ng = g_idx + over
        if (ng >= G).any() or (d - d0) >= 126:
            wins.append((d0, d - d0))
            d0 = d
            g_idx[:] = 0
            fill[:] = 0
            over = k > P
            assert not over.any()
            ng = g_idx
        fill = np.where(over, k, fill + k)
        g_idx = ng
        d += 1
    wins.append((d0, nsh - d0))
    return wins


def build_tiles(src_sorted, dst_sorted, lo, hi, wins, cfg):
    """Pack one core's dst-sorted edges into the global windows.
    Returns srcs [T,P,G] i32, sids [T,P,G] f32, mr [T,G*P] f32."""
    G = cfg.G
    SENT = cfg.npad  # gather sentinel: explicit zero row of the table
    counts = np.bincount(dst_sorted - lo, minlength=hi - lo)
    starts = np.zeros(hi - lo + 1, dtype=np.int64)
    np.cumsum(counts, out=starts[1:])

    T = len(wins)
    srcs = np.full((T, P, G), SENT, dtype=np.int32)
    sids = np.full((T, P, G), 127.0, dtype=np.float32)
    mr = np.full((T, G, P), 127.0, dtype=np.float32)
    for t, (d0, w) in enumerate(wins):
        g_idx, fill = 0, 0
        for dl in range(d0, d0 + w):
            k = int(counts[dl])
            if k == 0:
                continue
            if fill + k > P:
                g_idx += 1
                fill = 0
            assert g_idx < G
            sid = dl - d0
            s0 = int(starts[dl])
            sl = slice(fill, fill + k)
            srcs[t, sl, g_idx] = src_sorted[s0:s0 + k]
            sids[t, sl, g_idx] = float(sid)
            mr[t, g_idx, sl] = float(sid)
            fill += k
    return srcs, sids, mr.reshape(T, G * P)


def host_prep(x, edge_index, cfg):
    n = cfg.n_real
    src = np.concatenate([np.asarray(edge_index[0]), np.arange(n)]).astype(np.int64)
    dst = np.concatenate([np.asarray(edge_index[1]), np.arange(n)]).astype(np.int64)
    order = np.argsort(dst, kind="stable")
    src_s = src[order].astype(np.int32)
    dst_s = dst[order].astype(np.int32)

    xpad = np.zeros((cfg.npad, cfg.IN), dtype=BF16)
    xpad[:n] = np.asarray(x, dtype=np.float32).astype(BF16)

    bounds = np.searchsorted(dst_s, np.arange(0, cfg.npad + 1, cfg.nshard))
    deg = np.zeros((cfg.n_cores, cfg.nshard), dtype=np.int64)
    for c in range(cfg.n_cores):
        lo, hi = c * cfg.nshard, (c + 1) * cfg.nshard
        e0, e1 = bounds[c], bounds[c + 1]
        deg[c] = np.bincount(dst_s[e0:e1] - lo, minlength=cfg.nshard)

    wins = window_schedule(deg, cfg)
    cfg.T = len(wins)

    metas = []
    for c in range(cfg.n_cores):
        lo, hi = c * cfg.nshard, (c + 1) * cfg.nshard
        e0, e1 = bounds[c], bounds[c + 1]
        srcs, sids, mr = build_tiles(src_s[e0:e1], dst_s[e0:e1], lo, hi,
                                     wins, cfg)
        T, G = cfg.T, cfg.G
        # device layouts: srcs/sids -> [P, T*G] ; mr -> [Tpad(partition), G*P]
        srcs_dev = np.ascontiguousarray(
            srcs.transpose(1, 0, 2).reshape(P, T * G))
        sids_dev = np.ascontiguousarray(
            sids.transpose(1, 0, 2).reshape(P, T * G))
        metas.append((srcs_dev, sids_dev, mr.astype(BF16)))
    return xpad, wins, metas


def prep_weights(W1, a1_src, a1_dst, W2, a2_src, a2_dst, Wm1, bm1, b2, Wm2, cfg):
    def blockdiag(a, ch):
        B = np.zeros((cfg.A * ch, cfg.A), dtype=np.float32)
        for h in range(cfg.A):
            B[h * ch:(h + 1) * ch, h] = a[h]
        return B

    W1 = np.asarray(W1, np.float32)
    W2 = np.asarray(W2, np.float32)
    Wm1 = np.asarray(Wm1, np.float32)
    # [W | W bd(a_src) | W bd(a_dst)] — device uses cols [0:C+A) for the
    # gather table and cols [C+A:C+2A) for the shard-local alpha_dst table
    W1aug = np.concatenate(
        [W1, W1 @ blockdiag(np.asarray(a1_src, np.float32), cfg.CH1),
         W1 @ blockdiag(np.asarray(a1_dst, np.float32), cfg.CH1)], axis=1)
    W2aug = np.concatenate(
        [W2, W2 @ blockdiag(np.asarray(a2_src, np.float32), cfg.CH2),
         W2 @ blockdiag(np.asarray(a2_dst, np.float32), cfg.CH2)], axis=1)
    bm1p = np.asarray(bm1, np.float32) + np.asarray(b2, np.float32) @ Wm1
    return dict(
        W1aug=W1aug.astype(BF16),
        W2aug=W2aug.astype(BF16),
        Wm1=(Wm1 / cfg.A).astype(np.float32),  # folds the head-mean 1/8
        bm1=bm1p.reshape(-1, 1).astype(np.float32),
        Wm2=np.asarray(Wm2, np.float32),
    )


# ------------------------------------------------------------- device program

AMP_REPS = 1
DEBUG_OUTPUTS = False


def build_program(cfg, wins):
    from concourse import bass, bacc, mybir
    import concourse.tile as tile

    f32 = mybir.dt.float32
    bf16 = mybir.dt.bfloat16
    i32 = mybir.dt.int32
    A, C1, C2, K1, K2, G = cfg.A, cfg.C1, cfg.C2, cfg.K1, cfg.K2, cfg.G
    NSH, NPAD, T = cfg.nshard, cfg.npad, cfg.T
    NB = NSH // P       # 49 node blocks per shard
    SB = 512            # node-major superblock

    nc = bacc.Bacc("TRN2", target_bir_lowering=False, debug=False,
                   num_devices=cfg.n_cores, num_swdge_queues=4)

    xpad = nc.dram_tensor("xpad", [NPAD, cfg.IN], bf16, kind="ExternalInput")
    xshard = nc.dram_tensor("xshard", [NSH, cfg.IN], bf16,
                            kind="ExternalInput")
    srcs_d = nc.dram_tensor("srcs", [P, T * G], i32, kind="ExternalInput")
    sids_d = nc.dram_tensor("sids", [P, T * G], f32, kind="ExternalInput")
    mr_d = nc.dram_tensor("mr", [T, G * P], bf16, kind="ExternalInput")
    W1aug_d = nc.dram_tensor("W1aug", [cfg.IN, K1 + A], bf16,
                             kind="ExternalInput")
    W2aug_d = nc.dram_tensor("W2aug", [C1, K2 + A], bf16,
                             kind="ExternalInput")
    Wm1_d = nc.dram_tensor("Wm1", [64, 64], f32, kind="ExternalInput")
    bm1_d = nc.dram_tensor("bm1", [64, 1], f32, kind="ExternalInput")
    Wm2_d = nc.dram_tensor("Wm2", [64, 2], f32, kind="ExternalInput")

    hs1 = nc.dram_tensor("hs1", [NPAD + P, K1], bf16, kind="Internal")
    ad1t = nc.dram_tensor("ad1t", [NSH + P, A], bf16, kind="Internal")
    ad2t = nc.dram_tensor("ad2t", [NSH + P, A], bf16, kind="Internal")
    t2shard = nc.dram_tensor("t2shard", [NSH, C1], bf16, kind="Internal")
    h2shard = nc.dram_tensor("h2shard", [NSH, K2], bf16, kind="Internal")
    h2full = nc.dram_tensor("h2full", [NPAD + P, K2], bf16, kind="Internal",
                            addr_space="Shared")
    h2m = nc.dram_tensor("h2m", [NSH, 64], f32, kind="Internal")
    outf = nc.dram_tensor("outf", [2, NSH], f32, kind="ExternalOutput")
    if DEBUG_OUTPUTS:
        dbg_hs1 = nc.dram_tensor("dbg_hs1", [1024, K1], bf16,
                                 kind="ExternalOutput")
        dbg_ad1 = nc.dram_tensor("dbg_ad1", [NSH, A], bf16,
                                 kind="ExternalOutput")
        dbg_t2 = nc.dram_tensor("dbg_t2", [NSH, C1], bf16,
                                kind="ExternalOutput")
        dbg_h2s = nc.dram_tensor("dbg_h2s", [NSH, K2], bf16,
                                 kind="ExternalOutput")
        dbg_h2m = nc.dram_tensor("dbg_h2m", [NSH, 64], f32,
                                 kind="ExternalOutput")

    EXP = mybir.ActivationFunctionType.Exp
    RELU = mybir.ActivationFunctionType.Relu
    ABS = mybir.ActivationFunctionType.Abs
    EQ = mybir.AluOpType.is_equal
    MUL = mybir.AluOpType.mult
    ADD = mybir.AluOpType.add

    with tile.TileContext(nc) as tc:
        with (
            tc.tile_pool(name="const", bufs=1) as cp,
            tc.tile_pool(name="work", bufs=4) as wp,
        ):
            # ---- constants ----
            iota = cp.tile([P, 1], i32, tag="iotai")
            nc.gpsimd.iota(iota[:], pattern=[[0, 1]], base=0,
                           channel_multiplier=1)
            iotaf = cp.tile([P, 1], f32, tag="iotaf")
            nc.vector.tensor_copy(iotaf[:], iota[:])
            iotar = cp.tile([P, P], i32, tag="iotari")
            nc.gpsimd.iota(iotar[:], pattern=[[1, P]], base=0,
                           channel_multiplier=0)
            iotarf = cp.tile([P, P], f32, tag="iotarf")
            nc.vector.tensor_copy(iotarf[:], iotar[:])
            ones1 = cp.tile([1, P], bf16, tag="ones1")
            nc.gpsimd.memset(ones1[:], 1.0)
            zrow = cp.tile([P, K2], bf16, tag="zrow")
            nc.gpsimd.memset(zrow[:], 0.0)
            ident = cp.tile([P, P], f32, tag="ident")
            from concourse.masks import make_identity
            make_identity(nc, ident[:])

            w1aug = cp.tile([cfg.IN, K1 + A], bf16, tag="w1aug")
            nc.sync.dma_start(w1aug[:], W1aug_d[:])
            w2aug = [cp.tile([P, K2 + A], bf16, tag=f"w2aug{b}",
                             name=f"w2aug{b}") for b in range(2)]
            for b in range(2):
                nc.sync.dma_start(w2aug[b][:], W2aug_d[b * P:(b + 1) * P, :])
            wm1 = cp.tile([64, 64], f32, tag="wm1")
            nc.sync.dma_start(wm1[:], Wm1_d[:])
            bm1 = cp.tile([64, 1], f32, tag="bm1")
            nc.sync.dma_start(bm1[:], bm1_d[:])
            wm2 = cp.tile([64, 2], f32, tag="wm2")
            nc.sync.dma_start(wm2[:], Wm2_d[:])

            srcs = cp.tile([P, T * G], i32, tag="srcs")
            nc.sync.dma_start(srcs[:], srcs_d[:])
            sids = cp.tile([P, T * G], f32, tag="sids")
            nc.sync.dma_start(sids[:], sids_d[:])

            # zero pad rows of the gather tables (sentinel row NPAD..NPAD+P)
            nc.sync.dma_start(hs1[NPAD:NPAD + P, :], zrow[:, 0:K1])
            nc.sync.dma_start(h2full[NPAD:NPAD + P, :], zrow[:, 0:K2])
            nc.sync.dma_start(ad1t[NSH:NSH + P, :], zrow[:, 0:A])
            nc.sync.dma_start(ad2t[NSH:NSH + P, :], zrow[:, 0:A])

            for _rep in range(AMP_REPS):
              # ---- phase A0: ad1t = xshard @ W1ad (shard-local rows) ----
              with tc.tile_pool(name="psA0", bufs=4, space="PSUM") as psA0:
                  done = 0
                  while done < NSH:
                      cur = min(SB, NSH - done)
                      nbl = cur // P
                      xsT = wp.tile([P, cur], bf16, tag=f"a0_xsT{cur}")
                      nc.sync.dma_start(xsT[:], xshard[done:done + cur, :],
                                        transpose=True)
                      adc = wp.tile([P, nbl * A], bf16, tag=f"a0_adc{cur}")
                      for nb in range(nbl):
                          ps = psA0.tile([P, A], f32, tag="a0_ps")
                          nc.tensor.matmul(
                              ps[:], lhsT=xsT[:, nb * P:(nb + 1) * P],
                              rhs=w1aug[:, K1:K1 + A],
                              start=True, stop=True)
                          nc.any.tensor_copy(adc[:, nb * A:(nb + 1) * A],
                                             ps[:])
                      nc.sync.dma_start(
                          ad1t[done:done + cur, :].rearrange(
                              "(b p) c -> p b c", p=P),
                          adc[:].rearrange("p (b c) -> p b c", b=nbl))
                      done += cur

              # ---- phase A1: hs1[n] = x @ [W1 | W1 a1s | W1 a1d], all nodes
              with tc.tile_pool(name="psA1", bufs=4, space="PSUM") as psA:
                  for sb in range(NPAD // SB):
                      xT = wp.tile([P, SB], bf16, tag="a1_xT")
                      nc.sync.dma_start(xT[:], xpad[sb * SB:(sb + 1) * SB, :],
                                        transpose=True)
                      hcat = wp.tile([P, 4 * K1], bf16, tag="a1_hcat")
                      for nb in range(4):
                          ps = psA.tile([P, K1], f32, tag="a1_ps")
                          nc.tensor.matmul(ps[:], lhsT=xT[:, nb * P:(nb + 1) * P],
                                           rhs=w1aug[:, 0:K1],
                                           start=True, stop=True)
                          nc.any.tensor_copy(hcat[:, nb * K1:(nb + 1) * K1],
                                             ps[:])
                      nc.sync.dma_start(
                          hs1[sb * SB:(sb + 1) * SB, :].rearrange(
                              "(b p) c -> p b c", p=P),
                          hcat[:].rearrange("p (b c) -> p b c", b=4))

              def dbg_copy(dst, src, rows, cols, dt):
                  for i in range(rows // P):
                      c = wp.tile([P, cols], dt, tag=f"dbgc{cols}{dt}")
                      nc.sync.dma_start(c[:], src[i * P:(i + 1) * P, 0:cols])
                      nc.sync.dma_start(dst[i * P:(i + 1) * P, :], c[:])

              if DEBUG_OUTPUTS:
                  dbg_copy(dbg_hs1, hs1, 1024, K1, bf16)
                  dbg_copy(dbg_ad1, ad1t, NSH, A, bf16)

              # ---- edge-tile phase (shared between the two GAT layers) ----
              def edge_prefetch(layer, t):
                  KW = K1 if layer == 1 else K2
                  tbl = hs1 if layer == 1 else h2full
                  d0, w = wins[t]
                  gt = wp.tile([P, G * KW], bf16, tag=f"g{layer}",
                               name=f"g{layer}")
                  for g in range(G):
                      nc.gpsimd.indirect_dma_start(
                          out=gt[:, g * KW:(g + 1) * KW],
                          out_offset=None, in_=tbl[:],
                          in_offset=bass.IndirectOffsetOnAxis(
                              ap=srcs[:, t * G + g:t * G + g + 1], axis=0))
                  # alpha_dst rows for this window: shard-local rows d0..d0+P
                  adx = wp.tile([P, A], bf16, tag="adx")
                  adt = ad1t if layer == 1 else ad2t
                  nc.sync.dma_start(adx[:], adt[d0:d0 + P, :])
                  mr = wp.tile([1, G * P], bf16, tag="mrt")
                  nc.sync.dma_start(mr[:], mr_d[t:t + 1, :])
                  return dict(gt=gt, adx=adx, mr=mr, t=t)

              def edge_compute(layer, st, pools):
                  gt, adx, t = st["gt"], st["adx"], st["t"]
                  KW = K1 if layer == 1 else K2
                  CO = C1 if layer == 1 else C2
                  CH = cfg.CH1 if layer == 1 else cfg.CH2
                  d0, w = wins[t]

                  # segb[p, g*P+s] = sid of slot s in group g (broadcast rows)
                  segb = pools["seg"].tile([P, G * P], f32, tag="segps")
                  nc.tensor.matmul(segb[:], lhsT=ones1[:], rhs=st["mr"][:],
                                   start=True, stop=True)
                  # Eg[p, g*P+s] = (p == sid(g,s)) ; lhsT for alpha_dst expand
                  Eg = wp.tile([P, G * P], bf16, tag="Eg")
                  nc.vector.tensor_tensor(
                      out=Eg[:], in0=iotaf[:].to_broadcast((P, G * P)),
                      in1=segb[:], op=EQ)
                  # ETg[p, g*P+s'] = (sid_of_slot_p(g) == s') ; aggregation
                  ETg = wp.tile([P, G * P], bf16, tag="ETg")
                  sl = sids[:, t * G:(t + 1) * G]
                  in0 = bass.AP(sl.tensor, sl.offset,
                                [list(sl.ap[0]), [1, G], [0, P]])
                  in1 = bass.AP(iotarf.tensor, iotarf[:].offset,
                                [list(iotarf[:].ap[0]), [0, G], [1, P]])
                  out = bass.AP(ETg.tensor, ETg[:].offset,
                                [list(ETg[:].ap[0]), [P, G], [1, P]])
                  nc.vector.tensor_tensor(out=out, in0=in0, in1=in1, op=EQ)

                  # attention logits: alpha_src (gathered cols) + Eg @ adx
                  att = pools["att"].tile([P, G * A], f32, tag="attps")
                  for g in range(G):
                      nc.tensor.matmul(att[:, g * A:(g + 1) * A],
                                       lhsT=Eg[:, g * P:(g + 1) * P],
                                       rhs=adx[:], start=True, stop=True,
                                       skip_group_check=True)
                  asv = bass.AP(gt.tensor, gt[:].offset + CO,
                                [list(gt[:].ap[0]), [KW, G], [1, A]])
                  ex = wp.tile([P, G * A], f32, tag="ex")
                  exv = ex[:].rearrange("p (g a) -> p g a", g=G)
                  nc.vector.tensor_tensor(out=exv, in0=asv,
                                          in1=att[:].rearrange(
                                              "p (g a) -> p g a", g=G),
                                          op=ADD)
                  # leaky-relu as 0.6x + 0.4|x| (one PSUM input per op)
                  ab = wp.tile([P, G * A], f32, tag="ab")
                  nc.scalar.activation(ab[:], ex[:], ABS, scale=0.4)
                  nc.vector.scalar_tensor_tensor(
                      out=ex[:], in0=ex[:], scalar=0.6, in1=ab[:],
                      op0=MUL, op1=ADD)
                  nc.scalar.activation(ex[:], ex[:], EXP)

                  # M per group: [ex (A) | msg (CO)] ; msg = h * ex_broadcast
                  M = wp.tile([P, G * (A + CO)], bf16, tag=f"M{layer}",
                              name=f"M{layer}")
                  mex = bass.AP(M.tensor, M[:].offset,
                                [list(M[:].ap[0]), [A + CO, G], [1, A]])
                  nc.any.tensor_copy(mex, ex[:].rearrange(
                      "p (g a) -> p g a", g=G))
                  for g in range(G):
                      nc.vector.tensor_tensor(
                          out=M[:, g * (A + CO) + A:(g + 1) * (A + CO)]
                              .rearrange("p (h c) -> p h c", h=A),
                          in0=gt[:, g * KW:g * KW + CO]
                              .rearrange("p (h c) -> p h c", h=A),
                          in1=ex[:, g * A:(g + 1) * A][:, :, None]
                              .to_broadcast((P, A, CH)), op=MUL)

                  # aggregate: [denom | numer] += ETg_g^T @ M_g
                  if layer == 1:
                      pab = pools["pab"].tile([P, A + CO], f32, tag="pab")
                      for g in range(G):
                          nc.tensor.matmul(pab[:],
                                           lhsT=ETg[:, g * P:(g + 1) * P],
                                           rhs=M[:, g * (A + CO):
                                                 (g + 1) * (A + CO)],
                                           start=(g == 0), stop=(g == G - 1))
                      den = pab[:, 0:A]
                      num = pab[:, A:A + CO]
                  else:
                      HCO = CO // 2
                      pa = pools["pab"].tile([P, A + HCO], f32, tag="paL2")
                      pb = pools["pb"].tile([P, HCO], f32, tag="pbL2")
                      for g in range(G):
                          nc.tensor.matmul(pa[:],
                                           lhsT=ETg[:, g * P:(g + 1) * P],
                                           rhs=M[:, g * (A + CO):
                                                 g * (A + CO) + A + HCO],
                                           start=(g == 0), stop=(g == G - 1))
                          nc.tensor.matmul(pb[:],
                                           lhsT=ETg[:, g * P:(g + 1) * P],
                                           rhs=M[:, g * (A + CO) + A + HCO:
                                                 (g + 1) * (A + CO)],
                                           start=(g == 0), stop=(g == G - 1))
                      den = pa[:, 0:A]
                      num = None

                  # +eps so empty segments (zero-degree pad nodes) yield 0,
                  # not 0*inf=NaN — NaN rows would poison the Eg@adx matmul
                  # of later windows via 0*NaN.
                  r = wp.tile([P, A], f32, tag="r")
                  nc.vector.tensor_scalar(
                      out=r[:], in0=den, scalar1=1e-30, scalar2=None,
                      op0=ADD)
                  nc.vector.reciprocal(r[:], r[:])
                  if layer == 1:
                      h1r = wp.tile([P, C1], bf16, tag="h1r")
                      nc.vector.tensor_tensor(
                          out=h1r[:].rearrange("p (h c) -> p h c", h=A),
                          in0=num.rearrange("p (h c) -> p h c", h=A),
                          in1=r[:, :, None].to_broadcast((P, A, CH)), op=MUL)
                      nc.scalar.activation(h1r[:], h1r[:], RELU)
                      nc.sync.dma_start(t2shard[d0:d0 + w, :], h1r[0:w, :])
                  else:
                      tmp = wp.tile([P, C2], f32, tag="tmp2")
                      nc.vector.tensor_tensor(
                          out=tmp[:, 0:HCO].rearrange("p (h c) -> p h c",
                                                      h=A // 2),
                          in0=pa[:, A:A + HCO].rearrange("p (h c) -> p h c",
                                                         h=A // 2),
                          in1=r[:, 0:A // 2][:, :, None]
                              .to_broadcast((P, A // 2, CH)), op=MUL)
                      nc.vector.tensor_tensor(
                          out=tmp[:, HCO:CO].rearrange("p (h c) -> p h c",
                                                       h=A // 2),
                          in0=pb[:].rearrange("p (h c) -> p h c", h=A // 2),
                          in1=r[:, A // 2:A][:, :, None]
                              .to_broadcast((P, A // 2, CH)), op=MUL)
                      o2 = wp.tile([P, 64], f32, tag="o2")
                      cview = bass.AP(tmp.tensor, tmp[:].offset,
                                      [list(tmp[:].ap[0]), [1, 64], [64, A]])
                      nc.vector.tensor_reduce(
                          out=o2[:], in_=cview, axis=mybir.AxisListType.X,
                          op=ADD)
                      nc.sync.dma_start(h2m[d0:d0 + w, :], o2[0:w, :])

              # ---- phase I: layer-1 edge tiles ----
              with (
                  tc.tile_pool(name="psseg1", bufs=2, space="PSUM") as pseg,
                  tc.tile_pool(name="psatt1", bufs=2, space="PSUM") as patt,
                  tc.tile_pool(name="pspab1", bufs=2, space="PSUM") as ppab,
              ):
                  pools = dict(seg=pseg, att=patt, pab=ppab, pb=None)
                  pend = []
                  for t in range(T):
                      pend.append(edge_prefetch(1, t))
                      if len(pend) > 3:
                          edge_compute(1, pend.pop(0), pools)
                  for st in pend:
                      edge_compute(1, st, pools)

              if DEBUG_OUTPUTS:
                  dbg_copy(dbg_t2, t2shard, NSH, C1, bf16)

              # ---- phase A2: h2cat = t2 @ [W2 | W2 a2s | W2 a2d] (shard) ---
              with tc.tile_pool(name="psA2", bufs=4, space="PSUM") as psA2:
                  done = 0
                  while done < NSH:
                      cur = min(SB, NSH - done)
                      nbl = cur // P
                      t2T = [wp.tile([P, cur], bf16, tag=f"a2_t2T{b}_{cur}",
                                     name=f"a2_t2T{b}_{cur}")
                             for b in range(2)]
                      for b in range(2):
                          nc.sync.dma_start(
                              t2T[b][:],
                              t2shard[done:done + cur, b * P:(b + 1) * P],
                              transpose=True)
                      KF = K2 + A  # 528
                      hcat = wp.tile([P, nbl * KF], bf16, tag=f"a2_hcat{cur}")
                      for nb in range(nbl):
                          psa = psA2.tile([P, KF // 2], f32, tag="a2_psa")
                          psb = psA2.tile([P, KF // 2], f32, tag="a2_psb")
                          for b in range(2):
                              nc.tensor.matmul(
                                  psa[:], lhsT=t2T[b][:, nb * P:(nb + 1) * P],
                                  rhs=w2aug[b][:, 0:KF // 2],
                                  start=(b == 0), stop=(b == 1))
                              nc.tensor.matmul(
                                  psb[:], lhsT=t2T[b][:, nb * P:(nb + 1) * P],
                                  rhs=w2aug[b][:, KF // 2:KF],
                                  start=(b == 0), stop=(b == 1))
                          nc.any.tensor_copy(
                              hcat[:, nb * KF:nb * KF + KF // 2], psa[:])
                          nc.any.tensor_copy(
                              hcat[:, nb * KF + KF // 2:(nb + 1) * KF],
                              psb[:])
                      hc = hcat[:, 0:nbl * KF]
                      nc.sync.dma_start(
                          h2shard[done:done + cur, :].rearrange(
                              "(b p) c -> p b c", p=P),
                          bass.AP(hcat.tensor, hc.offset,
                                  [list(hc.ap[0]), [KF, nbl], [1, K2]]))
                      nc.sync.dma_start(
                          ad2t[done:done + cur, :].rearrange(
                              "(b p) c -> p b c", p=P),
                          bass.AP(hcat.tensor, hc.offset + K2,
                                  [list(hc.ap[0]), [KF, nbl], [1, A]]))
                      done += cur

              # ---- AllGather the h2cat node table ----
              if cfg.n_cores > 1:
                  nc.gpsimd.collective_compute(
                      "AllGather", mybir.AluOpType.bypass,
                      replica_groups=[list(range(cfg.n_cores))],
                      ins=[h2shard[0:NSH, :].opt()],
                      outs=[h2full[0:NPAD, :].opt()])
              else:
                  for i in range(NB):
                      cpt = wp.tile([P, K2], bf16, tag="cpt")
                      nc.sync.dma_start(cpt[:], h2shard[i * P:(i + 1) * P, :])
                      nc.sync.dma_start(h2full[i * P:(i + 1) * P, :], cpt[:])

              # ---- phase II: layer-2 edge tiles ----
              with (
                  tc.tile_pool(name="psseg2", bufs=2, space="PSUM") as pseg,
                  tc.tile_pool(name="psatt2", bufs=2, space="PSUM") as patt,
                  tc.tile_pool(name="pspa2", bufs=2, space="PSUM") as ppa,
                  tc.tile_pool(name="pspb2", bufs=2, space="PSUM") as ppb,
              ):
                  pools = dict(seg=pseg, att=patt, pab=ppa, pb=ppb)
                  pend = []
                  for t in range(T):
                      pend.append(edge_prefetch(2, t))
                      if len(pend) > 3:
                          edge_compute(2, pend.pop(0), pools)
                  for st in pend:
                      edge_compute(2, st, pools)

              if DEBUG_OUTPUTS:
                  dbg_copy(dbg_h2s, h2shard, NSH, K2, bf16)
                  dbg_copy(dbg_h2m, h2m, NSH, 64, f32)

              # ---- phase III: MLP node-major over the shard ----
              with tc.tile_pool(name="ps3", bufs=2, space="PSUM") as ps3:
                  for i in range(NB):
                      hm = wp.tile([P, 64], f32, tag="p3_hm")
                      nc.sync.dma_start(hm[:], h2m[i * P:(i + 1) * P, :])
                      tp = ps3.tile([64, P], f32, tag="tp64")
                      nc.tensor.transpose(tp[:], hm[:], ident[:])
                      hmT = wp.tile([64, P], f32, tag="p3_hmT")
                      nc.any.tensor_copy(hmT[:], tp[:])
                      m1 = ps3.tile([64, P], f32, tag="m1ps")
                      nc.tensor.matmul(m1[:], lhsT=wm1[:], rhs=hmT[:],
                                       start=True, stop=True)
                      hr = wp.tile([64, P], f32, tag="p3_hr")
                      nc.scalar.activation(hr[:], m1[:], RELU,
                                           bias=bm1[:, 0:1])
                      m2 = ps3.tile([2, P], f32, tag="m2ps")
                      nc.tensor.matmul(m2[:], lhsT=wm2[:], rhs=hr[:],
                                       start=True, stop=True)
                      ob = wp.tile([2, P], f32, tag="p3_ob")
                      nc.any.tensor_copy(ob[:], m2[:])
                      nc.sync.dma_start(outf[:, i * P:(i + 1) * P], ob[:])

    nc.compile()
    return nc


def make_in_maps(x, edge_index, weights, cfg):
    xpad, wins, metas = host_prep(x, edge_index, cfg)
    in_maps = []
    for c in range(cfg.n_cores):
        srcs_dev, sids_dev, mr_dev = metas[c]
        m = dict(
            xpad=xpad,
            xshard=np.ascontiguousarray(
                xpad[c * cfg.nshard:(c + 1) * cfg.nshard]),
            srcs=srcs_dev, sids=sids_dev, mr=mr_dev,
            **{k: np.ascontiguousarray(v) for k, v in weights.items()},
        )
        in_maps.append(m)
    return in_maps, wins


# ----------------------------------------------------------------- execution


class Runner:
    """Persistent jitted SPMD executor (mirrors bass2jax.run_bass_via_pjrt)."""

    def __init__(self, nc, n_cores):
        import jax
        from concourse import bass2jax, mybir
        from jax.sharding import Mesh, PartitionSpec, NamedSharding
        from jax.experimental.shard_map import shard_map

        bass2jax.install_neuronx_cc_hook()
        self.n_cores = n_cores
        self.jax = jax

        part_name = (nc.partition_id_tensor.name if nc.partition_id_tensor
                     else None)
        in_names, out_names, out_avals, zero_outs = [], [], [], []
        for alloc in nc.m.functions[0].allocations:
            if not isinstance(alloc, mybir.MemoryLocationSet):
                continue
            name = alloc.memorylocations[0].name
            if alloc.kind == "ExternalInput":
                if name != part_name:
                    in_names.append(name)
            elif alloc.kind == "ExternalOutput":
                shape = tuple(alloc.tensor_shape)
                dtype = mybir.dt.np(alloc.dtype)
                out_names.append(name)
                out_avals.append(jax.core.ShapedArray(shape, dtype))
                zero_outs.append(np.zeros(shape, dtype))
        self.in_names = list(in_names)
        self.out_names = out_names
        self.out_avals = out_avals
        self.zero_outs = zero_outs
        n_params = len(in_names)
        n_outs = len(out_names)
        all_names = in_names + out_names
        if part_name is not None:
            all_names.append(part_name)

        from concourse.bass2jax import _bass_exec_p, partition_id_tensor

        def _body(*args):
            operands = list(args)
            if part_name is not None:
                operands.append(partition_id_tensor())
            outs = _bass_exec_p.bind(
                *operands,
                out_avals=tuple(out_avals),
                in_names=tuple(all_names),
                out_names=tuple(out_names),
                lowering_input_output_aliases=(),
                sim_require_finite=False,
                sim_require_nnan=False,
                nc=nc,
            )
            return tuple(outs)

        devices = jax.devices()[:n_cores]
        self.mesh = Mesh(np.asarray(devices), ("core",))
        self.spec = NamedSharding(self.mesh, PartitionSpec("core"))
        in_specs = (PartitionSpec("core"),) * (n_params + n_outs)
        out_specs = (PartitionSpec("core"),) * n_outs
        donate = tuple(range(n_params, n_params + n_outs))
        self.fn = jax.jit(
            shard_map(_body, mesh=self.mesh, in_specs=in_specs,
                      out_specs=out_specs, check_rep=False),
            donate_argnums=donate, keep_unused=True)

    def put_inputs(self, in_maps):
        jax = self.jax
        self.dev_in = [
            jax.device_put(
                np.concatenate([np.asarray(in_maps[c][n])
                                for c in range(self.n_cores)], axis=0),
                self.spec)
            for n in self.in_names
        ]
        jax.block_until_ready(self.dev_in)

    def _zo(self):
        jax = self.jax
        zo = [jax.device_put(
            np.zeros((self.n_cores * z.shape[0], *z.shape[1:]), z.dtype),
            self.spec) for z in self.zero_outs]
        jax.block_until_ready(zo)
        return zo

    def run(self):
        jax = self.jax
        zo = self._zo()
        t0 = time.perf_counter_ns()
        outs = self.fn(*self.dev_in, *zo)
        jax.block_until_ready(outs)
        t1 = time.perf_counter_ns()
        res = {
            name: np.asarray(outs[i]).reshape(
                self.n_cores, *self.out_avals[i].shape)
            for i, name in enumerate(self.out_names)
        }
        return res, t1 - t0

    def chain(self, n):
        """Issue n pipelined executions; return total wall ns."""
        jax = self.jax
        zos = [self._zo() for _ in range(n)]
        t0 = time.perf_counter_ns()
        outs = [self.fn(*self.dev_in, *z) for z in zos]
        jax.block_until_ready(outs)
        t1 = time.perf_counter_ns()
        return t1 - t0

    def measure_exec_ns(self, n_lo=4, n_hi=16, trials=5):
        """Steady-state per-execution time: slope of wall time vs chain
        length (pipelined runs amortize the tunnel round-trip). Median of
        several trials for robustness against wall-clock noise."""
        slopes = []
        for _ in range(trials):
            t_hi = self.chain(n_hi)
            t_lo = self.chain(n_lo)
            slope = (t_hi - t_lo) / (n_hi - n_lo)
            if slope > 0:
                slopes.append(slope)
        if not slopes:
            return None
        return int(sorted(slopes)[len(slopes) // 2])


_CACHE = {}
LAST_EXEC_NS = None


def kernel(x, edge_index, W1, a1_src, a1_dst, b1, W2, a2_src, a2_dst, b2,
           Wm1, bm1, Wm2, bm2):
    global LAST_EXEC_NS
    assert float(np.abs(np.asarray(b1)).max()) == 0.0, \
        "nonzero b1 unsupported by this kernel build"

    cfg = Cfg(n_real=N_REAL, n_cores=N_CORES, nshard=NSHARD, T=0)
    weights = prep_weights(W1, a1_src, a1_dst, W2, a2_src, a2_dst,
                           Wm1, bm1, b2, Wm2, cfg)
    in_maps, wins = make_in_maps(x, edge_index, weights, cfg)

    key = ("prog", cfg.T, tuple(wins[:8]))
    if key not in _CACHE:
        nc = build_program(cfg, wins)
        _CACHE.clear()
        _CACHE[key] = Runner(nc, cfg.n_cores)
    runner = _CACHE[key]
    runner.put_inputs(in_maps)

    res, _ = runner.run()          # warm-up (includes compile on first call)
    res, dt = runner.run()         # correctness-bearing warm run
    ns = runner.measure_exec_ns()
    LAST_EXEC_NS = ns if ns is not None else dt

    out = res["outf"].transpose(0, 2, 1).reshape(cfg.npad, 2)[:cfg.n_real]
    return (out + np.asarray(bm2, np.float32)).astype(np.float32)
